# revision 1
# baseline (speedup 1.0000x reference)
"""Trainium2 Bass kernel for nn_AutoregressiveDecoder (GRU decoder w/ greedy argmax feedback).

B=64, L=128, E=512, H=512, V=32000, T=64, 8 NeuronCores.

Sharding: vocab (V) split 8 ways; each core holds its W_fc.T shard resident in
SBUF, computes the full GRU (replicated) in fp32, its logits shard, and its
local (max, argmax).  A per-step AllGather of the 8 (max, argmax) pairs gives
every core the global argmax; feedback x = emb[ids] comes from an indirect DMA
gather out of a full emb copy in each core's DRAM.

Self-contained: hardcodes shapes; only imports the platform toolchain.
"""
import sys

if "/opt/trn_rl_repo" not in sys.path:
    sys.path.insert(0, "/opt/trn_rl_repo")

import numpy as np

import concourse.bass as bass
import concourse.mybir as mybir
import concourse.bacc as bacc
import concourse.tile as tile
import concourse.bass_utils as bass_utils
from concourse.masks import make_identity

F32 = mybir.dt.float32
F32R = mybir.dt.float32r
U32 = mybir.dt.uint32
I32 = mybir.dt.int32
AF = mybir.ActivationFunctionType
OP = mybir.AluOpType
AX = mybir.AxisListType

B, L, E, H, V, T = 64, 128, 512, 512, 32000, 64
NC_N = 8
VS = V // NC_N          # 4000 vocab per core
VSP = 4096              # padded (8 tiles of 512)
KC = H // 128           # 4 contraction chunks
NVT = VSP // 512        # 8 vocab tiles per core
NEG = -1.0e30

# ---- build flags ----
LOGITS_F32R = True      # fast fp32r logits + exact fp32 top-3 re-eval
USE_MTAB = True         # gi via gather from host-precomputed emb @ W_ih.T


def _mm_acc(nc, out_ap, lhsT, rhs_list, start_first):
    """Accumulating matmul helper: sequence of (lhsT_ap, rhs_ap) into out."""
    n = len(rhs_list)
    for i, (lt, rh) in enumerate(rhs_list):
        nc.tensor.matmul(out_ap, lt, rh,
                         start=(start_first and i == 0), stop=(i == n - 1))


def build(t_steps=T, logits_f32r=LOGITS_F32R, use_mtab=USE_MTAB, no_cc=False):
    nc = bacc.Bacc("TRN2", target_bir_lowering=False, debug=False,
                   num_devices=NC_N)

    # ---------------- DRAM I/O ----------------
    d_emb = None
    if not use_mtab:
        d_emb = nc.dram_tensor("emb", [V, E], F32, kind="ExternalInput").ap()
    d_wihT = nc.dram_tensor("wihT", [H, 3 * H], F32, kind="ExternalInput").ap()
    d_whhT = nc.dram_tensor("whhT", [H, 3 * H], F32, kind="ExternalInput").ap()
    wfc_dt = F32R if logits_f32r else F32
    d_wfcT = nc.dram_tensor("wfcT", [H, VSP], wfc_dt, kind="ExternalInput").ap()
    d_wprojT = nc.dram_tensor("wprojT", [L, H], F32, kind="ExternalInput").ap()
    d_zT = nc.dram_tensor("zT", [L, B], F32, kind="ExternalInput").ap()
    d_bias_gi = nc.dram_tensor("bias_gi", [1, 3 * H], F32, kind="ExternalInput").ap()
    d_bias_hn = nc.dram_tensor("bias_hn", [1, H], F32, kind="ExternalInput").ap()
    d_bias_fc = nc.dram_tensor("bias_fc", [1, VSP], wfc_dt, kind="ExternalInput").ap()
    d_bias_proj = nc.dram_tensor("bias_proj", [1, H], F32, kind="ExternalInput").ap()
    d_rank = nc.dram_tensor("rank_col", [B, 1], F32, kind="ExternalInput").ap()
    if use_mtab:
        d_mtab_rz = nc.dram_tensor("mtab_rz", [V, 1024], F32, kind="ExternalInput").ap()
        d_mtab_n = nc.dram_tensor("mtab_n", [V, 512], F32, kind="ExternalInput").ap()
    if logits_f32r:
        # per-core shard of [W_fc | b_fc] for exact candidate re-evaluation
        d_wb = nc.dram_tensor("wb", [VS, E + 1], F32, kind="ExternalInput").ap()
    d_out = nc.dram_tensor("out", [B, t_steps * VS], F32, kind="ExternalOutput").ap()

    with tile.TileContext(nc) as tc:
        with tc.tile_pool(name="wts", bufs=1) as wpool, \
             tc.tile_pool(name="sb", bufs=2) as sb, \
             tc.tile_pool(name="sb1", bufs=1) as sb1, \
             tc.tile_pool(name="lgps", bufs=2, space="PSUM") as lgp, \
             tc.tile_pool(name="grups", bufs=1, space="PSUM") as grup, \
             tc.tile_pool(name="tps", bufs=2, space="PSUM") as tps, \
             tc.tile_pool(name="dr", bufs=2, space="DRAM") as dr:
            # ---------------- load weights ----------------
            wih = wpool.tile([128, KC * 3 * H], F32)          # 4x[128,1536]
            whh = wpool.tile([128, KC * 3 * H], F32)
            wfc = wpool.tile([128, KC * VSP], wfc_dt)         # 4x[128,4096]
            wproj = wpool.tile([128, H], F32)
            zT_sb = wpool.tile([128, B], F32)
            for k in range(KC):
                nc.sync.dma_start(wih[:, k * 3 * H:(k + 1) * 3 * H],
                                  d_wihT[k * 128:(k + 1) * 128, :])
                nc.sync.dma_start(whh[:, k * 3 * H:(k + 1) * 3 * H],
                                  d_whhT[k * 128:(k + 1) * 128, :])
                nc.sync.dma_start(wfc[:, k * VSP:(k + 1) * VSP],
                                  d_wfcT[k * 128:(k + 1) * 128, :])
            nc.sync.dma_start(wproj[:], d_wprojT)
            nc.sync.dma_start(zT_sb[:], d_zT)
            b_gi = wpool.tile([1, 3 * H], F32)
            b_hn = wpool.tile([1, H], F32)
            b_fc = wpool.tile([1, VSP], wfc_dt)
            b_proj = wpool.tile([1, H], F32)
            rank_col = wpool.tile([B, 1], F32)
            nc.sync.dma_start(b_gi[:], d_bias_gi)
            nc.sync.dma_start(b_hn[:], d_bias_hn)
            nc.sync.dma_start(b_fc[:], d_bias_fc)
            nc.sync.dma_start(b_proj[:], d_bias_proj)
            nc.sync.dma_start(rank_col[:], d_rank)
            ident = wpool.tile([B, B], F32)
            make_identity(nc, ident[:])
            ones1 = wpool.tile([1, 128], F32)
            nc.vector.memset(ones1[:], 1.0)

            if logits_f32r:
                ones_r = wpool.tile([1, 128], F32R)
                nc.vector.tensor_copy(ones_r[:], ones1[:])
                cand8 = wpool.tile([B, 8], F32)
                nc.vector.memset(cand8[:], NEG)
                cpair = wpool.tile([B, 8], F32)
                nc.vector.memset(cpair[:], NEG)

            # ---------------- h0 ----------------
            h0_ps = lgp.tile([B, H], F32, tag="lg")
            nc.tensor.matmul(h0_ps[:], zT_sb[:], wproj[:], start=True, stop=False)
            nc.tensor.matmul(h0_ps[:], ones1[0:1, 0:B], b_proj[:],
                             start=False, stop=True)
            h_cur = sb.tile([B, H], F32, tag="h")
            nc.scalar.copy(h_cur[:], h0_ps[:])

            # transposed h (lhsT layout): [128, KC*64], chunk k at [:, 64k:64k+64]
            def transpose_to(dst_sb, src_ap, extra_dst=None):
                tp = tps.tile([128, 256], F32, tag="tp")
                for k in range(KC):
                    nc.tensor.transpose(tp[:, k * 64:(k + 1) * 64],
                                        src_ap[:, k * 128:(k + 1) * 128],
                                        ident[:])
                nc.scalar.copy(dst_sb[:], tp[:])
                if extra_dst is not None:
                    nc.vector.tensor_copy(extra_dst[:], tp[:])

            hT = sb.tile([128, KC * 64], F32, tag="hT")
            if logits_f32r:
                hT_r = sb.tile([128, KC * 64], F32R, tag="hTr")
                transpose_to(hT, h_cur[:], extra_dst=hT_r)
            else:
                hT_r = None
                transpose_to(hT, h_cur[:])

            xT = hT            # step 0: x = h0
            x_sb = None
            ids_i32 = None

            # DRAM bounce tiles for the collective
            for t in range(t_steps):
                # ---------- gh (+ rz biases) : psum ----------
                mtab_step = use_mtab and t > 0
                rz_ps = grup.tile([B, 1024], F32, tag="rz")
                ghn_ps = grup.tile([B, 512], F32, tag="ghn")
                # rz region: gh first (start); gi mms accumulate on top unless
                # this is an mtab step (gi arrives via gather + DVE add).
                for j in range(2):
                    o = rz_ps[:, j * 512:(j + 1) * 512]
                    for k in range(KC):
                        nc.tensor.matmul(o, hT[:, k * 64:(k + 1) * 64],
                                         whh[:, k * 3 * H + j * 512:
                                             k * 3 * H + (j + 1) * 512],
                                         start=(k == 0),
                                         stop=(mtab_step and k == KC - 1))
                # ghn = (h @ W_hh.T)_n + b_hh_n
                for k in range(KC):
                    nc.tensor.matmul(ghn_ps[:], hT[:, k * 64:(k + 1) * 64],
                                     whh[:, k * 3 * H + 1024:k * 3 * H + 1536],
                                     start=(k == 0), stop=False)
                nc.tensor.matmul(ghn_ps[:], ones1[0:1, 0:B], b_hn[:],
                                 start=False, stop=True)

                # ---------- gi ----------
                if mtab_step:
                    # rz: copy gh_rz to SBUF (hidden), then CCE-add gather of
                    # mtab's rz slice lands gi_rz + gh_rz in one DMA.
                    rz_acc = sb.tile([B, 1024], F32, tag="rzacc")
                    nc.scalar.copy(rz_acc[:], rz_ps[:])
                    nc.gpsimd.indirect_dma_start(
                        out=rz_acc[:], out_offset=None, in_=d_mtab_rz,
                        in_offset=bass.IndirectOffsetOnAxis(ap=ids_i32[:, 0:1], axis=0),
                        compute_op=OP.add)
                    gin_sb = sb.tile([B, H], F32, tag="ginsb")
                    nc.gpsimd.indirect_dma_start(
                        out=gin_sb[:], out_offset=None, in_=d_mtab_n,
                        in_offset=bass.IndirectOffsetOnAxis(ap=ids_i32[:, 0:1], axis=0))
                else:
                    gin_ps = grup.tile([B, 512], F32, tag="gin")
                    for j in range(2):
                        o = rz_ps[:, j * 512:(j + 1) * 512]
                        for k in range(KC):
                            nc.tensor.matmul(o, xT[:, k * 64:(k + 1) * 64],
                                             wih[:, k * 3 * H + j * 512:
                                                 k * 3 * H + (j + 1) * 512],
                                             start=False, stop=False)
                        nc.tensor.matmul(o, ones1[0:1, 0:B],
                                         b_gi[:, j * 512:(j + 1) * 512],
                                         start=False, stop=True)
                    for k in range(KC):
                        nc.tensor.matmul(gin_ps[:], xT[:, k * 64:(k + 1) * 64],
                                         wih[:, k * 3 * H + 1024:k * 3 * H + 1536],
                                         start=(k == 0), stop=False)
                    nc.tensor.matmul(gin_ps[:], ones1[0:1, 0:B],
                                     b_gi[:, 1024:1536], start=False, stop=True)

                # ---------- gates ----------
                rz_sb = sb1.tile([B, 1024], F32, tag="rzsb")
                if mtab_step:
                    nc.scalar.activation(rz_sb[:], rz_acc[:], AF.Sigmoid)
                else:
                    nc.scalar.activation(rz_sb[:], rz_ps[:], AF.Sigmoid)
                u_sb = sb1.tile([B, H], F32, tag="u")
                nc.vector.tensor_tensor(out=u_sb[:], in0=rz_sb[:, 0:512],
                                        in1=ghn_ps[:], op=OP.mult)
                if mtab_step:
                    nc.vector.tensor_tensor(out=u_sb[:], in0=u_sb[:],
                                            in1=gin_sb[:], op=OP.add)
                else:
                    nc.vector.tensor_tensor(out=u_sb[:], in0=u_sb[:],
                                            in1=gin_ps[:], op=OP.add)
                n_sb = sb1.tile([B, H], F32, tag="n")
                nc.scalar.activation(n_sb[:], u_sb[:], AF.Tanh)
                f1_sb = sb1.tile([B, H], F32, tag="f1")
                nc.vector.tensor_tensor(out=f1_sb[:], in0=rz_sb[:, 512:1024],
                                        in1=h_cur[:], op=OP.mult)
                f2_sb = sb1.tile([B, H], F32, tag="f2")
                nc.vector.tensor_scalar(out=f2_sb[:], in0=rz_sb[:, 512:1024],
                                        scalar1=-1.0, scalar2=1.0,
                                        op0=OP.mult, op1=OP.add)
                h_new = sb.tile([B, H], F32, tag="h")
                nc.vector.tensor_tensor(out=h_new[:], in0=f2_sb[:], in1=n_sb[:],
                                        op=OP.mult)
                nc.vector.tensor_tensor(out=h_new[:], in0=h_new[:], in1=f1_sb[:],
                                        op=OP.add)
                h_cur = h_new

                # ---------- hT ----------
                hT = sb.tile([128, KC * 64], F32, tag="hT")
                if logits_f32r:
                    hT_r = sb.tile([128, KC * 64], F32R, tag="hTr")
                    transpose_to(hT, h_cur[:], extra_dst=hT_r)
                else:
                    transpose_to(hT, h_cur[:])

                # ---------- logits ----------
                lg_sb = sb1.tile([B, VSP], F32, tag="lg_sb")
                tm8 = sb1.tile([B, NVT * 8], F32, tag="tm8")
                lg_lhs = hT_r if logits_f32r else hT
                lg_ones = ones_r if logits_f32r else ones1
                for v in range(NVT):
                    lg_ps = lgp.tile([B, 512], F32, tag="lg")
                    for k in range(KC):
                        nc.tensor.matmul(
                            lg_ps[:], lg_lhs[:, k * 64:(k + 1) * 64],
                            wfc[:, k * VSP + v * 512:k * VSP + (v + 1) * 512],
                            start=(k == 0), stop=False)
                    nc.tensor.matmul(lg_ps[:], lg_ones[0:1, 0:B],
                                     b_fc[:, v * 512:(v + 1) * 512],
                                     start=False, stop=True)
                    nc.scalar.copy(lg_sb[:, v * 512:(v + 1) * 512], lg_ps[:])
                    nc.vector.max(out=tm8[:, v * 8:(v + 1) * 8],
                                  in_=lg_sb[:, v * 512:(v + 1) * 512])

                # DMA logits out (write only the real 4000)
                nc.sync.dma_start(d_out[:, t * VS:(t + 1) * VS], lg_sb[:, 0:VS])

                if t == t_steps - 1:
                    break       # no feedback needed after last step

                # ---------- local argmax ----------
                gmax = sb1.tile([B, 1], F32, tag="gmax")
                nc.vector.tensor_reduce(out=gmax[:], in_=tm8[:], axis=AX.X,
                                        op=OP.max)
                mi8 = sb1.tile([B, 8], U32, tag="mi8")
                if logits_f32r:
                    # fp32r logits are ~1.7e-4 loose; find top-3 candidates
                    # from the approximate logits, then re-evaluate them
                    # exactly in fp32 to pick the true argmax.
                    nc.vector.tensor_copy(cand8[:, 0:1], gmax[:])
                    v2t = sb1.tile([B, NVT * 8], F32, tag="v2t")
                    nc.vector.match_replace(out=v2t[:], in_to_replace=cand8[:, 0:8],
                                            in_values=tm8[:], imm_value=NEG)
                    v2v = sb1.tile([B, 1], F32, tag="v2v")
                    nc.vector.tensor_reduce(out=v2v[:], in_=v2t[:], axis=AX.X,
                                            op=OP.max)
                    nc.vector.tensor_copy(cpair[:, 0:1], v2v[:])
                    v3t = sb1.tile([B, NVT * 8], F32, tag="v3t")
                    nc.vector.match_replace(out=v3t[:], in_to_replace=cpair[:, 0:8],
                                            in_values=v2t[:], imm_value=NEG)
                    v3v = sb1.tile([B, 1], F32, tag="v3v")
                    nc.vector.tensor_reduce(out=v3v[:], in_=v3t[:], axis=AX.X,
                                            op=OP.max)
                    cand3 = sb1.tile([B, 8], F32, tag="cand3")
                    nc.vector.memset(cand3[:], NEG)
                    nc.vector.tensor_copy(cand3[:, 0:1], gmax[:])
                    nc.vector.tensor_copy(cand3[:, 1:2], v2v[:])
                    nc.vector.tensor_copy(cand3[:, 2:3], v3v[:])
                    nc.vector.max_index(out=mi8[:], in_max=cand3[:],
                                        in_values=lg_sb[:])
                    idl = sb1.tile([B, 8], I32, tag="idl")
                    nc.vector.tensor_copy(idl[:, 0:3], mi8[:, 0:3])
                    wb3 = sb1.tile([B, 3 * (E + 1)], F32, tag="wb3")
                    p3 = sb1.tile([B, 3 * E], F32, tag="p3")
                    for j in range(3):
                        nc.gpsimd.indirect_dma_start(
                            out=wb3[:, j * (E + 1):(j + 1) * (E + 1)],
                            out_offset=None, in_=d_wb,
                            in_offset=bass.IndirectOffsetOnAxis(
                                ap=idl[:, j:j + 1], axis=0))
                        nc.vector.tensor_tensor(
                            out=p3[:, j * E:(j + 1) * E], in0=h_cur[:],
                            in1=wb3[:, j * (E + 1):j * (E + 1) + E], op=OP.mult)
                    e3 = sb1.tile([B, 4], F32, tag="e3")
                    nc.vector.tensor_reduce(
                        out=e3[:, 0:3],
                        in_=p3[:].rearrange("p (j e) -> p j e", j=3),
                        axis=AX.X, op=OP.add)
                    # add per-candidate bias (wb3 col E of each 513-block)
                    nc.vector.tensor_tensor(
                        out=e3[:, 0:3], in0=e3[:, 0:3],
                        in1=wb3[:].rearrange("p (j e) -> p j e", j=3)[:, :, E:E + 1].squeeze(),
                        op=OP.add)
                    idf = sb1.tile([B, 8], F32, tag="idf")
                    nc.vector.tensor_copy(idf[:, 0:3], mi8[:, 0:3])
                    nc.vector.tensor_scalar(out=idf[:, 0:3], in0=idf[:, 0:3],
                                            scalar1=rank_col[:, 0:1],
                                            scalar2=None, op0=OP.add)
                    cmp01 = sb1.tile([B, 1], I32, tag="cmp01")
                    nc.vector.tensor_tensor(out=cmp01[:], in0=e3[:, 1:2],
                                            in1=e3[:, 0:1], op=OP.is_gt)
                    m01 = sb1.tile([B, 1], F32, tag="m01")
                    nc.vector.tensor_tensor(out=m01[:], in0=e3[:, 0:1],
                                            in1=e3[:, 1:2], op=OP.max)
                    g01 = sb1.tile([B, 1], F32, tag="g01")
                    nc.vector.select(out=g01[:], mask=cmp01[:],
                                     on_true=idf[:, 1:2], on_false=idf[:, 0:1])
                    cmp2 = sb1.tile([B, 1], I32, tag="cmp2")
                    nc.vector.tensor_tensor(out=cmp2[:], in0=e3[:, 2:3],
                                            in1=m01[:], op=OP.is_gt)
                    emax = sb1.tile([B, 1], F32, tag="emaxv")
                    nc.vector.tensor_tensor(out=emax[:], in0=m01[:],
                                            in1=e3[:, 2:3], op=OP.max)
                    gidf = sb1.tile([B, 1], F32, tag="gidf")
                    nc.vector.select(out=gidf[:], mask=cmp2[:],
                                     on_true=idf[:, 2:3], on_false=g01[:])
                else:
                    gmax8 = sb1.tile([B, 8], F32, tag="gmax8")
                    nc.vector.tensor_copy(gmax8[:], gmax[:].to_broadcast([B, 8]))
                    nc.vector.max_index(out=mi8[:], in_max=gmax8[:],
                                        in_values=lg_sb[:])
                    emax = gmax
                    gidf = sb1.tile([B, 1], F32, tag="gidf")
                    nc.vector.tensor_copy(gidf[:], mi8[:, 0:1])
                    nc.vector.tensor_scalar(out=gidf[:], in0=gidf[:],
                                            scalar1=rank_col[:, 0:1], scalar2=None,
                                            op0=OP.add)

                # ---------- AllGather of (emax, gid) ----------
                pay = sb1.tile([B, 2], F32, tag="pay")
                nc.vector.tensor_copy(pay[:, 0:1], emax[:])
                nc.vector.tensor_copy(pay[:, 1:2], gidf[:])
                payT_ps = tps.tile([128, 256], F32, tag="tp")
                nc.tensor.transpose(payT_ps[0:1, 0:64], pay[:, 0:1], ident[:])
                nc.tensor.transpose(payT_ps[0:1, 64:128], pay[:, 1:2], ident[:])
                pay_row = sb1.tile([1, 128], F32, tag="payrow")
                nc.vector.tensor_copy(pay_row[:], payT_ps[0:1, 0:128])
                cc_in = dr.tile([1, 128], F32, tag="ccin")
                cc_out = dr.tile([NC_N, 128], F32, tag="ccout")
                nc.gpsimd.dma_start(cc_in[:], pay_row[:])
                if no_cc:
                    # perf probe only (wrong results): skip the collective,
                    # replicate own payload into all 8 rows
                    for rr in range(NC_N):
                        nc.gpsimd.dma_start(cc_out[rr:rr + 1, :], cc_in[:])
                else:
                    nc.gpsimd.collective_compute(
                        "AllGather", OP.bypass,
                        replica_groups=[list(range(NC_N))],
                        ins=[cc_in[:].opt()], outs=[cc_out[:].opt()])
                ag_sb = sb1.tile([NC_N, 128], F32, tag="agsb")
                nc.gpsimd.dma_start(ag_sb[:], cc_out[:])

                # ---------- global argmax ----------
                agT_ps = tps.tile([128, 256], F32, tag="tp")
                nc.tensor.transpose(agT_ps[0:B, 0:8], ag_sb[:, 0:64],
                                    ident[0:8, 0:8])
                nc.tensor.transpose(agT_ps[0:B, 8:16], ag_sb[:, 64:128],
                                    ident[0:8, 0:8])
                agT = sb1.tile([B, 16], F32, tag="agTsb")
                nc.vector.tensor_copy(agT[:], agT_ps[0:B, 0:16])
                gm = sb1.tile([B, 1], F32, tag="gm")
                nc.vector.tensor_reduce(out=gm[:], in_=agT[:, 0:8], axis=AX.X,
                                        op=OP.max)
                mask = sb1.tile([B, 8], F32, tag="mask")
                nc.vector.tensor_scalar(out=mask[:], in0=agT[:, 0:8],
                                        scalar1=gm[:, 0:1], scalar2=None,
                                        op0=OP.is_lt)    # 1.0 where NOT max
                sel = sb1.tile([B, 8], F32, tag="sel")
                # sel = gid + (not-max)*BIG  -> min over ranks = winning gid
                nc.vector.tensor_scalar(out=sel[:], in0=mask[:],
                                        scalar1=2.0e9, scalar2=None,
                                        op0=OP.mult)
                nc.vector.tensor_tensor(out=sel[:], in0=sel[:], in1=agT[:, 8:16],
                                        op=OP.add)
                widf = sb1.tile([B, 1], F32, tag="widf")
                nc.vector.tensor_reduce(out=widf[:], in_=sel[:], axis=AX.X,
                                        op=OP.min)
                ids_i32 = sb1.tile([B, 1], I32, tag="ids")
                nc.vector.tensor_copy(ids_i32[:], widf[:])

                # ---------- feedback gather + xT ----------
                if not use_mtab:
                    x_sb = sb.tile([B, E], F32, tag="x")
                    nc.gpsimd.indirect_dma_start(
                        out=x_sb[:], out_offset=None, in_=d_emb,
                        in_offset=bass.IndirectOffsetOnAxis(ap=ids_i32[:, 0:1],
                                                            axis=0))
                    xT = sb.tile([128, KC * 64], F32, tag="xT")
                    transpose_to(xT, x_sb[:])

    nc.compile()
    return nc


_BUILT = {}


def _get_nc():
    key = (T, LOGITS_F32R, USE_MTAB)
    if key not in _BUILT:
        _BUILT[key] = build(T, LOGITS_F32R, USE_MTAB)
    return _BUILT[key]


def make_in_maps(z, emb, W_proj, b_proj, W_ih, b_ih, W_hh, b_hh, W_fc, b_fc):
    z = np.asarray(z, np.float32)
    emb = np.ascontiguousarray(np.asarray(emb, np.float32))
    W_proj = np.asarray(W_proj, np.float32)
    W_ih = np.asarray(W_ih, np.float32)
    W_hh = np.asarray(W_hh, np.float32)
    W_fc = np.asarray(W_fc, np.float32)
    b_proj = np.asarray(b_proj, np.float32)
    b_ih = np.asarray(b_ih, np.float32)
    b_hh = np.asarray(b_hh, np.float32)
    b_fc = np.asarray(b_fc, np.float32)

    wihT = np.ascontiguousarray(W_ih.T)            # [512, 1536]
    whhT = np.ascontiguousarray(W_hh.T)
    wprojT = np.ascontiguousarray(W_proj.T)        # [128, 512]
    zT = np.ascontiguousarray(z.T)                 # [128, 64]
    bias_gi = b_ih.copy()
    bias_gi[0:1024] += b_hh[0:1024]
    bias_gi = bias_gi[None, :]                     # [1, 1536]
    bias_hn = b_hh[None, 1024:1536]
    bias_proj = b_proj[None, :]

    common = dict(wihT=wihT, whhT=whhT, wprojT=wprojT, zT=zT,
                  bias_gi=bias_gi, bias_hn=bias_hn, bias_proj=bias_proj)
    if USE_MTAB:
        mtab = (emb @ W_ih.T + bias_gi).astype(np.float32)
        common["mtab_rz"] = np.ascontiguousarray(mtab[:, 0:1024])
        common["mtab_n"] = np.ascontiguousarray(mtab[:, 1024:1536])
    else:
        common["emb"] = emb

    in_maps = []
    for c in range(NC_N):
        wfc_sh = W_fc[c * VS:(c + 1) * VS, :]          # [4000, 512]
        wfcT = np.zeros((H, VSP), np.float32)
        wfcT[:, 0:VS] = wfc_sh.T
        bias_fc = np.full((1, VSP), NEG, np.float32)
        bias_fc[0, 0:VS] = b_fc[c * VS:(c + 1) * VS]
        rank_col = np.full((B, 1), float(c * VS), np.float32)
        m = dict(common)
        m.update(wfcT=np.ascontiguousarray(wfcT), bias_fc=bias_fc,
                 rank_col=rank_col)
        if LOGITS_F32R:
            m["wb"] = np.ascontiguousarray(
                np.concatenate([wfc_sh, b_fc[c * VS:(c + 1) * VS, None]],
                               axis=1))
        in_maps.append(m)
    return in_maps


def kernel(z, emb, W_proj, b_proj, W_ih, b_ih, W_hh, b_hh, W_fc, b_fc,
           context_length):
    assert int(context_length) == T
    nc = _get_nc()
    in_maps = make_in_maps(z, emb, W_proj, b_proj, W_ih, b_ih, W_hh, b_hh,
                           W_fc, b_fc)
    res = bass_utils.run_bass_kernel_spmd(nc, in_maps,
                                          core_ids=list(range(NC_N)))
    shards = [res.results[c]["out"].reshape(B, T, VS) for c in range(NC_N)]
    return np.concatenate(shards, axis=2)



# revision 6
# speedup vs baseline: 1.0697x; 1.0697x over previous
"""Trainium2 Bass kernel for nn_AutoregressiveDecoder (GRU decoder w/ greedy argmax feedback).

B=64, L=128, E=512, H=512, V=32000, T=64, 8 NeuronCores.

Sharding: vocab (V) split 8 ways; each core holds its W_fc.T shard resident in
SBUF, computes the full GRU (replicated) in fp32, its logits shard, and its
local (max, argmax).  A per-step AllGather of the 8 (max, argmax) pairs gives
every core the global argmax; feedback x = emb[ids] comes from an indirect DMA
gather out of a full emb copy in each core's DRAM.

Self-contained: hardcodes shapes; only imports the platform toolchain.
"""
import sys

if "/opt/trn_rl_repo" not in sys.path:
    sys.path.insert(0, "/opt/trn_rl_repo")

import numpy as np

import concourse.bass as bass
import concourse.mybir as mybir
import concourse.bacc as bacc
import concourse.tile as tile
import concourse.bass_utils as bass_utils
from concourse.masks import make_identity

F32 = mybir.dt.float32
F32R = mybir.dt.float32r
U32 = mybir.dt.uint32
I32 = mybir.dt.int32
AF = mybir.ActivationFunctionType
OP = mybir.AluOpType
AX = mybir.AxisListType

B, L, E, H, V, T = 64, 128, 512, 512, 32000, 64
NC_N = 8
VS = V // NC_N          # 4000 vocab per core
VSP = 4096              # padded (8 tiles of 512)
KC = H // 128           # 4 contraction chunks
NVT = VSP // 512        # 8 vocab tiles per core
NEG = -1.0e30

# ---- build flags ----
LOGITS_F32R = True      # fast fp32r logits + exact fp32 top-3 re-eval
USE_MTAB = True         # gi via gather from host-precomputed emb @ W_ih.T


def _mm_acc(nc, out_ap, lhsT, rhs_list, start_first):
    """Accumulating matmul helper: sequence of (lhsT_ap, rhs_ap) into out."""
    n = len(rhs_list)
    for i, (lt, rh) in enumerate(rhs_list):
        nc.tensor.matmul(out_ap, lt, rh,
                         start=(start_first and i == 0), stop=(i == n - 1))


def build(t_steps=T, logits_f32r=LOGITS_F32R, use_mtab=USE_MTAB, no_cc=False,
          probe=None):
    # probe (perf-only, wrong results):
    #   "noargmax": skip local/global argmax + exchange; ids forced to 0
    #   "nogather": noargmax + mtab indirect gathers -> regular row-0 DMAs
    #   "gruonly":  nogather + skip logits matmuls (dummy output writes)
    nc = bacc.Bacc("TRN2", target_bir_lowering=False, debug=False,
                   num_devices=NC_N)

    # ---------------- DRAM I/O ----------------
    d_emb = None
    if not use_mtab:
        d_emb = nc.dram_tensor("emb", [V, E], F32, kind="ExternalInput").ap()
    d_wihT = nc.dram_tensor("wihT", [H, 3 * H], F32, kind="ExternalInput").ap()
    d_whhT = nc.dram_tensor("whhT", [H, 3 * H], F32, kind="ExternalInput").ap()
    wfc_dt = F32R if logits_f32r else F32
    d_wfcT = nc.dram_tensor("wfcT", [H, VSP], wfc_dt, kind="ExternalInput").ap()
    d_wprojT = nc.dram_tensor("wprojT", [L, H], F32, kind="ExternalInput").ap()
    d_zT = nc.dram_tensor("zT", [L, B], F32, kind="ExternalInput").ap()
    d_bias_gi = nc.dram_tensor("bias_gi", [1, 3 * H], F32, kind="ExternalInput").ap()
    d_bias_hn = nc.dram_tensor("bias_hn", [1, H], F32, kind="ExternalInput").ap()
    d_bias_fc = nc.dram_tensor("bias_fc", [1, VSP], wfc_dt, kind="ExternalInput").ap()
    d_bias_proj = nc.dram_tensor("bias_proj", [1, H], F32, kind="ExternalInput").ap()
    d_rank = nc.dram_tensor("rank_col", [B, 1], F32, kind="ExternalInput").ap()
    if use_mtab:
        d_mtab_rz = nc.dram_tensor("mtab_rz", [V, 1024], F32, kind="ExternalInput").ap()
        d_mtab_n = nc.dram_tensor("mtab_n", [V, 512], F32, kind="ExternalInput").ap()
    if logits_f32r:
        # per-core shard of [W_fc | b_fc] for exact candidate re-evaluation
        d_wb = nc.dram_tensor("wb", [VS, E + 1], F32, kind="ExternalInput").ap()
    d_out = nc.dram_tensor("out", [B, t_steps * VS], F32, kind="ExternalOutput").ap()

    with tile.TileContext(nc) as tc:
        with tc.tile_pool(name="wts", bufs=1) as wpool, \
             tc.tile_pool(name="sb", bufs=2) as sb, \
             tc.tile_pool(name="sb1", bufs=1) as sb1, \
             tc.tile_pool(name="lgps", bufs=2, space="PSUM") as lgp, \
             tc.tile_pool(name="grups", bufs=1, space="PSUM") as grup, \
             tc.tile_pool(name="tps", bufs=2, space="PSUM") as tps, \
             tc.tile_pool(name="dr", bufs=2, space="DRAM") as dr:
            # ---------------- load weights ----------------
            wih = wpool.tile([128, KC * 3 * H], F32)          # 4x[128,1536]
            whh = wpool.tile([128, KC * 3 * H], F32)
            wfc = wpool.tile([128, KC * VSP], wfc_dt)         # 4x[128,4096]
            wproj = wpool.tile([128, H], F32)
            zT_sb = wpool.tile([128, B], F32)
            for k in range(KC):
                nc.sync.dma_start(wih[:, k * 3 * H:(k + 1) * 3 * H],
                                  d_wihT[k * 128:(k + 1) * 128, :])
                nc.sync.dma_start(whh[:, k * 3 * H:(k + 1) * 3 * H],
                                  d_whhT[k * 128:(k + 1) * 128, :])
                nc.sync.dma_start(wfc[:, k * VSP:(k + 1) * VSP],
                                  d_wfcT[k * 128:(k + 1) * 128, :])
            nc.sync.dma_start(wproj[:], d_wprojT)
            nc.sync.dma_start(zT_sb[:], d_zT)
            b_gi = wpool.tile([1, 3 * H], F32)
            b_hn = wpool.tile([1, H], F32)
            b_fc = wpool.tile([1, VSP], wfc_dt)
            b_proj = wpool.tile([1, H], F32)
            rank_col = wpool.tile([B, 1], F32)
            nc.sync.dma_start(b_gi[:], d_bias_gi)
            nc.sync.dma_start(b_hn[:], d_bias_hn)
            nc.sync.dma_start(b_fc[:], d_bias_fc)
            nc.sync.dma_start(b_proj[:], d_bias_proj)
            nc.sync.dma_start(rank_col[:], d_rank)
            ident = wpool.tile([B, B], F32)
            make_identity(nc, ident[:])
            ones1 = wpool.tile([1, 128], F32)
            nc.vector.memset(ones1[:], 1.0)

            if logits_f32r:
                ones_r = wpool.tile([1, 128], F32R)
                nc.vector.tensor_copy(ones_r[:], ones1[:])
                cand8 = wpool.tile([B, 8], F32)
                nc.vector.memset(cand8[:], NEG)
                cpair = wpool.tile([B, 8], F32)
                nc.vector.memset(cpair[:], NEG)

            # ---------------- h0 ----------------
            h0_ps = lgp.tile([B, H], F32, tag="lg")
            nc.tensor.matmul(h0_ps[:], zT_sb[:], wproj[:], start=True, stop=False)
            nc.tensor.matmul(h0_ps[:], ones1[0:1, 0:B], b_proj[:],
                             start=False, stop=True)
            h_cur = sb.tile([B, H], F32, tag="h")
            nc.scalar.copy(h_cur[:], h0_ps[:])

            # transposed h (lhsT layout): [128, KC*64], chunk k at [:, 64k:64k+64]
            def transpose_to(dst_sb, src_ap, extra_dst=None):
                tp = tps.tile([128, 256], F32, tag="tp")
                for k in range(KC):
                    nc.tensor.transpose(tp[:, k * 64:(k + 1) * 64],
                                        src_ap[:, k * 128:(k + 1) * 128],
                                        ident[:])
                nc.scalar.copy(dst_sb[:], tp[:])
                if extra_dst is not None:
                    nc.vector.tensor_copy(extra_dst[:], tp[:])

            hT = sb.tile([128, KC * 64], F32, tag="hT")
            if logits_f32r:
                hT_r = sb.tile([128, KC * 64], F32R, tag="hTr")
                transpose_to(hT, h_cur[:], extra_dst=hT_r)
            else:
                hT_r = None
                transpose_to(hT, h_cur[:])

            xT = hT            # step 0: x = h0
            x_sb = None
            ids_i32 = None
            if probe is not None:
                ids_i32 = wpool.tile([B, 1], I32)
                nc.vector.memset(ids_i32[:], 0.0)
                lg_dummy = wpool.tile([B, VS], F32)
                nc.vector.memset(lg_dummy[:], 0.0)

            # DRAM bounce tiles for the collective
            for t in range(t_steps):
                # ---------- gh (+ rz biases) : psum ----------
                mtab_step = use_mtab and t > 0
                rz_ps = grup.tile([B, 1024], F32, tag="rz")
                ghn_ps = grup.tile([B, 512], F32, tag="ghn")
                # rz region: gh first (start); gi mms accumulate on top unless
                # this is an mtab step (gi arrives via gather + DVE add).
                for j in range(2):
                    o = rz_ps[:, j * 512:(j + 1) * 512]
                    for k in range(KC):
                        nc.tensor.matmul(o, hT[:, k * 64:(k + 1) * 64],
                                         whh[:, k * 3 * H + j * 512:
                                             k * 3 * H + (j + 1) * 512],
                                         start=(k == 0),
                                         stop=(mtab_step and k == KC - 1))
                # ghn = (h @ W_hh.T)_n + b_hh_n
                for k in range(KC):
                    nc.tensor.matmul(ghn_ps[:], hT[:, k * 64:(k + 1) * 64],
                                     whh[:, k * 3 * H + 1024:k * 3 * H + 1536],
                                     start=(k == 0), stop=False)
                nc.tensor.matmul(ghn_ps[:], ones1[0:1, 0:B], b_hn[:],
                                 start=False, stop=True)

                # ---------- gi ----------
                if mtab_step:
                    # rz: copy gh_rz to SBUF (hidden), then CCE-add gather of
                    # mtab's rz slice lands gi_rz + gh_rz in one DMA.
                    rz_acc = sb.tile([B, 1024], F32, tag="rzacc")
                    nc.scalar.copy(rz_acc[:], rz_ps[:])
                    gin_sb = sb.tile([B, H], F32, tag="ginsb")
                    if probe in ("nogather", "gruonly"):
                        rz_tmp = sb.tile([B, 1024], F32, tag="rztmp")
                        nc.sync.dma_start(rz_tmp[:], d_mtab_rz[0:B, :])
                        nc.vector.tensor_tensor(out=rz_acc[:], in0=rz_acc[:],
                                                in1=rz_tmp[:], op=OP.add)
                        nc.sync.dma_start(gin_sb[:], d_mtab_n[0:B, :])
                    else:
                        nc.gpsimd.indirect_dma_start(
                            out=rz_acc[:], out_offset=None, in_=d_mtab_rz,
                            in_offset=bass.IndirectOffsetOnAxis(ap=ids_i32[:, 0:1], axis=0),
                            compute_op=OP.add)
                        nc.gpsimd.indirect_dma_start(
                            out=gin_sb[:], out_offset=None, in_=d_mtab_n,
                            in_offset=bass.IndirectOffsetOnAxis(ap=ids_i32[:, 0:1], axis=0))
                else:
                    gin_ps = grup.tile([B, 512], F32, tag="gin")
                    for j in range(2):
                        o = rz_ps[:, j * 512:(j + 1) * 512]
                        for k in range(KC):
                            nc.tensor.matmul(o, xT[:, k * 64:(k + 1) * 64],
                                             wih[:, k * 3 * H + j * 512:
                                                 k * 3 * H + (j + 1) * 512],
                                             start=False, stop=False)
                        nc.tensor.matmul(o, ones1[0:1, 0:B],
                                         b_gi[:, j * 512:(j + 1) * 512],
                                         start=False, stop=True)
                    for k in range(KC):
                        nc.tensor.matmul(gin_ps[:], xT[:, k * 64:(k + 1) * 64],
                                         wih[:, k * 3 * H + 1024:k * 3 * H + 1536],
                                         start=(k == 0), stop=False)
                    nc.tensor.matmul(gin_ps[:], ones1[0:1, 0:B],
                                     b_gi[:, 1024:1536], start=False, stop=True)

                # ---------- gates ----------
                rz_sb = sb1.tile([B, 1024], F32, tag="rzsb")
                if mtab_step:
                    nc.scalar.activation(rz_sb[:], rz_acc[:], AF.Sigmoid)
                else:
                    nc.scalar.activation(rz_sb[:], rz_ps[:], AF.Sigmoid)
                u_sb = sb1.tile([B, H], F32, tag="u")
                nc.vector.tensor_tensor(out=u_sb[:], in0=rz_sb[:, 0:512],
                                        in1=ghn_ps[:], op=OP.mult)
                if mtab_step:
                    nc.vector.tensor_tensor(out=u_sb[:], in0=u_sb[:],
                                            in1=gin_sb[:], op=OP.add)
                else:
                    nc.vector.tensor_tensor(out=u_sb[:], in0=u_sb[:],
                                            in1=gin_ps[:], op=OP.add)
                n_sb = sb1.tile([B, H], F32, tag="n")
                nc.scalar.activation(n_sb[:], u_sb[:], AF.Tanh)
                f1_sb = sb1.tile([B, H], F32, tag="f1")
                nc.vector.tensor_tensor(out=f1_sb[:], in0=rz_sb[:, 512:1024],
                                        in1=h_cur[:], op=OP.mult)
                f2_sb = sb1.tile([B, H], F32, tag="f2")
                nc.vector.tensor_scalar(out=f2_sb[:], in0=rz_sb[:, 512:1024],
                                        scalar1=-1.0, scalar2=1.0,
                                        op0=OP.mult, op1=OP.add)
                h_new = sb.tile([B, H], F32, tag="h")
                nc.vector.tensor_tensor(out=h_new[:], in0=f2_sb[:], in1=n_sb[:],
                                        op=OP.mult)
                nc.vector.tensor_tensor(out=h_new[:], in0=h_new[:], in1=f1_sb[:],
                                        op=OP.add)
                h_cur = h_new

                # ---------- hT ----------
                hT = sb.tile([128, KC * 64], F32, tag="hT")
                if logits_f32r:
                    hT_r = sb.tile([128, KC * 64], F32R, tag="hTr")
                    transpose_to(hT, h_cur[:], extra_dst=hT_r)
                else:
                    transpose_to(hT, h_cur[:])

                # ---------- logits ----------
                if probe == "gruonly":
                    nc.sync.dma_start(d_out[:, t * VS:(t + 1) * VS], lg_dummy[:])
                    continue
                lg_sb = sb1.tile([B, VSP], F32, tag="lg_sb")
                tm8 = sb1.tile([B, NVT * 8], F32, tag="tm8")
                lg_lhs = hT_r if logits_f32r else hT
                lg_ones = ones_r if logits_f32r else ones1
                for v in range(NVT):
                    lg_ps = lgp.tile([B, 512], F32, tag="lg")
                    for k in range(KC):
                        nc.tensor.matmul(
                            lg_ps[:], lg_lhs[:, k * 64:(k + 1) * 64],
                            wfc[:, k * VSP + v * 512:k * VSP + (v + 1) * 512],
                            start=(k == 0), stop=False)
                    nc.tensor.matmul(lg_ps[:], lg_ones[0:1, 0:B],
                                     b_fc[:, v * 512:(v + 1) * 512],
                                     start=False, stop=True)
                    nc.scalar.copy(lg_sb[:, v * 512:(v + 1) * 512], lg_ps[:])
                    nc.vector.max(out=tm8[:, v * 8:(v + 1) * 8],
                                  in_=lg_sb[:, v * 512:(v + 1) * 512])

                # DMA logits out (write only the real 4000)
                nc.sync.dma_start(d_out[:, t * VS:(t + 1) * VS], lg_sb[:, 0:VS])

                if t == t_steps - 1:
                    break       # no feedback needed after last step

                if probe is not None:
                    continue    # ids stay 0; skip argmax + exchange

                # ---------- local argmax ----------
                gmax = sb1.tile([B, 1], F32, tag="gmax")
                nc.vector.tensor_reduce(out=gmax[:], in_=tm8[:], axis=AX.X,
                                        op=OP.max)
                mi8 = sb1.tile([B, 8], U32, tag="mi8")
                if logits_f32r:
                    # fp32r logits are ~1.7e-4 loose; find top-3 candidates
                    # from the approximate logits, then re-evaluate them
                    # exactly in fp32 to pick the true argmax.
                    nc.vector.tensor_copy(cand8[:, 0:1], gmax[:])
                    v2t = sb1.tile([B, NVT * 8], F32, tag="v2t")
                    nc.vector.match_replace(out=v2t[:], in_to_replace=cand8[:, 0:8],
                                            in_values=tm8[:], imm_value=NEG)
                    v2v = sb1.tile([B, 1], F32, tag="v2v")
                    nc.vector.tensor_reduce(out=v2v[:], in_=v2t[:], axis=AX.X,
                                            op=OP.max)
                    nc.vector.tensor_copy(cpair[:, 0:1], v2v[:])
                    v3t = sb1.tile([B, NVT * 8], F32, tag="v3t")
                    nc.vector.match_replace(out=v3t[:], in_to_replace=cpair[:, 0:8],
                                            in_values=v2t[:], imm_value=NEG)
                    v3v = sb1.tile([B, 1], F32, tag="v3v")
                    nc.vector.tensor_reduce(out=v3v[:], in_=v3t[:], axis=AX.X,
                                            op=OP.max)
                    cand3 = sb1.tile([B, 8], F32, tag="cand3")
                    nc.vector.memset(cand3[:], NEG)
                    nc.vector.tensor_copy(cand3[:, 0:1], gmax[:])
                    nc.vector.tensor_copy(cand3[:, 1:2], v2v[:])
                    nc.vector.tensor_copy(cand3[:, 2:3], v3v[:])
                    nc.vector.max_index(out=mi8[:], in_max=cand3[:],
                                        in_values=lg_sb[:])
                    idl = sb1.tile([B, 8], I32, tag="idl")
                    nc.vector.tensor_copy(idl[:, 0:3], mi8[:, 0:3])
                    wb3 = sb1.tile([B, 3 * (E + 1)], F32, tag="wb3")
                    p3 = sb1.tile([B, 3 * E], F32, tag="p3")
                    for j in range(3):
                        nc.gpsimd.indirect_dma_start(
                            out=wb3[:, j * (E + 1):(j + 1) * (E + 1)],
                            out_offset=None, in_=d_wb,
                            in_offset=bass.IndirectOffsetOnAxis(
                                ap=idl[:, j:j + 1], axis=0))
                        nc.vector.tensor_tensor(
                            out=p3[:, j * E:(j + 1) * E], in0=h_cur[:],
                            in1=wb3[:, j * (E + 1):j * (E + 1) + E], op=OP.mult)
                    e3 = sb1.tile([B, 4], F32, tag="e3")
                    nc.vector.tensor_reduce(
                        out=e3[:, 0:3],
                        in_=p3[:].rearrange("p (j e) -> p j e", j=3),
                        axis=AX.X, op=OP.add)
                    # add per-candidate bias (wb3 col E of each 513-block)
                    nc.vector.tensor_tensor(
                        out=e3[:, 0:3], in0=e3[:, 0:3],
                        in1=wb3[:].rearrange("p (j e) -> p j e", j=3)[:, :, E:E + 1].squeeze(),
                        op=OP.add)
                    idf = sb1.tile([B, 8], F32, tag="idf")
                    nc.vector.tensor_copy(idf[:, 0:3], mi8[:, 0:3])
                    nc.vector.tensor_scalar(out=idf[:, 0:3], in0=idf[:, 0:3],
                                            scalar1=rank_col[:, 0:1],
                                            scalar2=None, op0=OP.add)
                    cmp01 = sb1.tile([B, 1], I32, tag="cmp01")
                    nc.vector.tensor_tensor(out=cmp01[:], in0=e3[:, 1:2],
                                            in1=e3[:, 0:1], op=OP.is_gt)
                    m01 = sb1.tile([B, 1], F32, tag="m01")
                    nc.vector.tensor_tensor(out=m01[:], in0=e3[:, 0:1],
                                            in1=e3[:, 1:2], op=OP.max)
                    g01 = sb1.tile([B, 1], F32, tag="g01")
                    nc.vector.select(out=g01[:], mask=cmp01[:],
                                     on_true=idf[:, 1:2], on_false=idf[:, 0:1])
                    cmp2 = sb1.tile([B, 1], I32, tag="cmp2")
                    nc.vector.tensor_tensor(out=cmp2[:], in0=e3[:, 2:3],
                                            in1=m01[:], op=OP.is_gt)
                    emax = sb1.tile([B, 1], F32, tag="emaxv")
                    nc.vector.tensor_tensor(out=emax[:], in0=m01[:],
                                            in1=e3[:, 2:3], op=OP.max)
                    gidf = sb1.tile([B, 1], F32, tag="gidf")
                    nc.vector.select(out=gidf[:], mask=cmp2[:],
                                     on_true=idf[:, 2:3], on_false=g01[:])
                else:
                    gmax8 = sb1.tile([B, 8], F32, tag="gmax8")
                    nc.vector.tensor_copy(gmax8[:], gmax[:].to_broadcast([B, 8]))
                    nc.vector.max_index(out=mi8[:], in_max=gmax8[:],
                                        in_values=lg_sb[:])
                    emax = gmax
                    gidf = sb1.tile([B, 1], F32, tag="gidf")
                    nc.vector.tensor_copy(gidf[:], mi8[:, 0:1])
                    nc.vector.tensor_scalar(out=gidf[:], in0=gidf[:],
                                            scalar1=rank_col[:, 0:1], scalar2=None,
                                            op0=OP.add)

                # ---------- AllGather of (emax, gid) ----------
                pay = sb1.tile([B, 2], F32, tag="pay")
                nc.vector.tensor_copy(pay[:, 0:1], emax[:])
                nc.vector.tensor_copy(pay[:, 1:2], gidf[:])
                payT_ps = tps.tile([128, 256], F32, tag="tp")
                nc.tensor.transpose(payT_ps[0:1, 0:64], pay[:, 0:1], ident[:])
                nc.tensor.transpose(payT_ps[0:1, 64:128], pay[:, 1:2], ident[:])
                pay_row = sb1.tile([1, 128], F32, tag="payrow")
                nc.vector.tensor_copy(pay_row[:], payT_ps[0:1, 0:128])
                cc_in = dr.tile([1, 128], F32, tag="ccin")
                cc_out = dr.tile([NC_N, 128], F32, tag="ccout")
                nc.gpsimd.dma_start(cc_in[:], pay_row[:])
                if no_cc:
                    # perf probe only (wrong results): skip the collective,
                    # replicate own payload into all 8 rows
                    for rr in range(NC_N):
                        nc.gpsimd.dma_start(cc_out[rr:rr + 1, :], cc_in[:])
                else:
                    nc.gpsimd.collective_compute(
                        "AllGather", OP.bypass,
                        replica_groups=[list(range(NC_N))],
                        ins=[cc_in[:].opt()], outs=[cc_out[:].opt()])
                ag_sb = sb1.tile([NC_N, 128], F32, tag="agsb")
                nc.gpsimd.dma_start(ag_sb[:], cc_out[:])

                # ---------- global argmax ----------
                agT_ps = tps.tile([128, 256], F32, tag="tp")
                nc.tensor.transpose(agT_ps[0:B, 0:8], ag_sb[:, 0:64],
                                    ident[0:8, 0:8])
                nc.tensor.transpose(agT_ps[0:B, 8:16], ag_sb[:, 64:128],
                                    ident[0:8, 0:8])
                agT = sb1.tile([B, 16], F32, tag="agTsb")
                nc.vector.tensor_copy(agT[:], agT_ps[0:B, 0:16])
                gm = sb1.tile([B, 1], F32, tag="gm")
                nc.vector.tensor_reduce(out=gm[:], in_=agT[:, 0:8], axis=AX.X,
                                        op=OP.max)
                mask = sb1.tile([B, 8], F32, tag="mask")
                nc.vector.tensor_scalar(out=mask[:], in0=agT[:, 0:8],
                                        scalar1=gm[:, 0:1], scalar2=None,
                                        op0=OP.is_lt)    # 1.0 where NOT max
                sel = sb1.tile([B, 8], F32, tag="sel")
                # sel = gid + (not-max)*BIG  -> min over ranks = winning gid
                nc.vector.tensor_scalar(out=sel[:], in0=mask[:],
                                        scalar1=2.0e9, scalar2=None,
                                        op0=OP.mult)
                nc.vector.tensor_tensor(out=sel[:], in0=sel[:], in1=agT[:, 8:16],
                                        op=OP.add)
                widf = sb1.tile([B, 1], F32, tag="widf")
                nc.vector.tensor_reduce(out=widf[:], in_=sel[:], axis=AX.X,
                                        op=OP.min)
                ids_i32 = sb1.tile([B, 1], I32, tag="ids")
                nc.vector.tensor_copy(ids_i32[:], widf[:])

                # ---------- feedback gather + xT ----------
                if not use_mtab:
                    x_sb = sb.tile([B, E], F32, tag="x")
                    nc.gpsimd.indirect_dma_start(
                        out=x_sb[:], out_offset=None, in_=d_emb,
                        in_offset=bass.IndirectOffsetOnAxis(ap=ids_i32[:, 0:1],
                                                            axis=0))
                    xT = sb.tile([128, KC * 64], F32, tag="xT")
                    transpose_to(xT, x_sb[:])

    nc.compile()
    return nc


_BUILT = {}


def _get_nc():
    key = (T, LOGITS_F32R, USE_MTAB)
    if key not in _BUILT:
        _BUILT[key] = build(T, LOGITS_F32R, USE_MTAB)
    return _BUILT[key]


def make_in_maps(z, emb, W_proj, b_proj, W_ih, b_ih, W_hh, b_hh, W_fc, b_fc):
    z = np.asarray(z, np.float32)
    emb = np.ascontiguousarray(np.asarray(emb, np.float32))
    W_proj = np.asarray(W_proj, np.float32)
    W_ih = np.asarray(W_ih, np.float32)
    W_hh = np.asarray(W_hh, np.float32)
    W_fc = np.asarray(W_fc, np.float32)
    b_proj = np.asarray(b_proj, np.float32)
    b_ih = np.asarray(b_ih, np.float32)
    b_hh = np.asarray(b_hh, np.float32)
    b_fc = np.asarray(b_fc, np.float32)

    wihT = np.ascontiguousarray(W_ih.T)            # [512, 1536]
    whhT = np.ascontiguousarray(W_hh.T)
    wprojT = np.ascontiguousarray(W_proj.T)        # [128, 512]
    zT = np.ascontiguousarray(z.T)                 # [128, 64]
    bias_gi = b_ih.copy()
    bias_gi[0:1024] += b_hh[0:1024]
    bias_gi = bias_gi[None, :]                     # [1, 1536]
    bias_hn = b_hh[None, 1024:1536]
    bias_proj = b_proj[None, :]

    common = dict(wihT=wihT, whhT=whhT, wprojT=wprojT, zT=zT,
                  bias_gi=bias_gi, bias_hn=bias_hn, bias_proj=bias_proj)
    if USE_MTAB:
        mtab = (emb @ W_ih.T + bias_gi).astype(np.float32)
        common["mtab_rz"] = np.ascontiguousarray(mtab[:, 0:1024])
        common["mtab_n"] = np.ascontiguousarray(mtab[:, 1024:1536])
    else:
        common["emb"] = emb

    in_maps = []
    for c in range(NC_N):
        wfc_sh = W_fc[c * VS:(c + 1) * VS, :]          # [4000, 512]
        wfcT = np.zeros((H, VSP), np.float32)
        wfcT[:, 0:VS] = wfc_sh.T
        bias_fc = np.full((1, VSP), NEG, np.float32)
        bias_fc[0, 0:VS] = b_fc[c * VS:(c + 1) * VS]
        rank_col = np.full((B, 1), float(c * VS), np.float32)
        m = dict(common)
        m.update(wfcT=np.ascontiguousarray(wfcT), bias_fc=bias_fc,
                 rank_col=rank_col)
        if LOGITS_F32R:
            m["wb"] = np.ascontiguousarray(
                np.concatenate([wfc_sh, b_fc[c * VS:(c + 1) * VS, None]],
                               axis=1))
        in_maps.append(m)
    return in_maps


def kernel(z, emb, W_proj, b_proj, W_ih, b_ih, W_hh, b_hh, W_fc, b_fc,
           context_length):
    assert int(context_length) == T
    nc = _get_nc()
    in_maps = make_in_maps(z, emb, W_proj, b_proj, W_ih, b_ih, W_hh, b_hh,
                           W_fc, b_fc)
    res = bass_utils.run_bass_kernel_spmd(nc, in_maps,
                                          core_ids=list(range(NC_N)))
    shards = [res.results[c]["out"].reshape(B, T, VS) for c in range(NC_N)]
    return np.concatenate(shards, axis=2)



# revision 7
# speedup vs baseline: 4.3536x; 4.0701x over previous
"""Trainium2 Bass kernel for nn_AutoregressiveDecoder (GRU decoder w/ greedy argmax feedback).

B=64, L=128, E=512, H=512, V=32000, T=64, 8 NeuronCores.

Sharding: vocab (V) split 8 ways; each core holds its W_fc.T shard resident in
SBUF, computes the full GRU (replicated) in fp32, its logits shard, and its
local (max, argmax).  A per-step AllGather of the 8 (max, argmax) pairs gives
every core the global argmax; feedback x = emb[ids] comes from an indirect DMA
gather out of a full emb copy in each core's DRAM.

Self-contained: hardcodes shapes; only imports the platform toolchain.
"""
import sys

if "/opt/trn_rl_repo" not in sys.path:
    sys.path.insert(0, "/opt/trn_rl_repo")

import numpy as np

import concourse.bass as bass
import concourse.mybir as mybir
import concourse.bacc as bacc
import concourse.tile as tile
import concourse.bass_utils as bass_utils
from concourse.masks import make_identity

F32 = mybir.dt.float32
F32R = mybir.dt.float32r
U32 = mybir.dt.uint32
I32 = mybir.dt.int32
AF = mybir.ActivationFunctionType
OP = mybir.AluOpType
AX = mybir.AxisListType

B, L, E, H, V, T = 64, 128, 512, 512, 32000, 64
NC_N = 8
VS = V // NC_N          # 4000 vocab per core
VSP = 4096              # padded (8 tiles of 512)
KC = H // 128           # 4 contraction chunks
NVT = VSP // 512        # 8 vocab tiles per core
NEG = -1.0e30

# ---- build flags ----
LOGITS_F32R = True      # fast fp32r logits + exact fp32 top-3 re-eval
USE_MTAB = True         # gi via gather from host-precomputed emb @ W_ih.T


def _mm_acc(nc, out_ap, lhsT, rhs_list, start_first):
    """Accumulating matmul helper: sequence of (lhsT_ap, rhs_ap) into out."""
    n = len(rhs_list)
    for i, (lt, rh) in enumerate(rhs_list):
        nc.tensor.matmul(out_ap, lt, rh,
                         start=(start_first and i == 0), stop=(i == n - 1))


def build(t_steps=T, logits_f32r=LOGITS_F32R, use_mtab=USE_MTAB, no_cc=False,
          probe=None):
    # probe (perf-only, wrong results):
    #   "noargmax": skip local/global argmax + exchange; ids forced to 0
    #   "nogather": noargmax + mtab indirect gathers -> regular row-0 DMAs
    #   "gruonly":  nogather + skip logits matmuls (dummy output writes)
    nc = bacc.Bacc("TRN2", target_bir_lowering=False, debug=False,
                   num_devices=NC_N)

    # ---------------- DRAM I/O ----------------
    d_emb = None
    if not use_mtab:
        d_emb = nc.dram_tensor("emb", [V, E], F32, kind="ExternalInput").ap()
    d_wihT = nc.dram_tensor("wihT", [H, 3 * H], F32, kind="ExternalInput").ap()
    d_whhT = nc.dram_tensor("whhT", [H, 3 * H], F32, kind="ExternalInput").ap()
    wfc_dt = F32R if logits_f32r else F32
    d_wfcT = nc.dram_tensor("wfcT", [H, VSP], wfc_dt, kind="ExternalInput").ap()
    d_wprojT = nc.dram_tensor("wprojT", [L, H], F32, kind="ExternalInput").ap()
    d_zT = nc.dram_tensor("zT", [L, B], F32, kind="ExternalInput").ap()
    d_bias_gi = nc.dram_tensor("bias_gi", [1, 3 * H], F32, kind="ExternalInput").ap()
    d_bias_hn = nc.dram_tensor("bias_hn", [1, H], F32, kind="ExternalInput").ap()
    d_bias_fc = nc.dram_tensor("bias_fc", [1, VSP], wfc_dt, kind="ExternalInput").ap()
    d_bias_proj = nc.dram_tensor("bias_proj", [1, H], F32, kind="ExternalInput").ap()
    d_rank = nc.dram_tensor("rank_col", [B, 1], F32, kind="ExternalInput").ap()
    if use_mtab:
        d_mtab_rz = nc.dram_tensor("mtab_rz", [V, 1024], F32, kind="ExternalInput").ap()
        d_mtab_n = nc.dram_tensor("mtab_n", [V, 512], F32, kind="ExternalInput").ap()
    if logits_f32r:
        # per-core shard of [W_fc | b_fc] for exact candidate re-evaluation
        d_wb = nc.dram_tensor("wb", [VS, E + 1], F32, kind="ExternalInput").ap()
    d_out = nc.dram_tensor("out", [B, t_steps * VS], F32, kind="ExternalOutput").ap()

    with tile.TileContext(nc) as tc:
        with tc.tile_pool(name="wts", bufs=1) as wpool, \
             tc.tile_pool(name="sb", bufs=2) as sb, \
             tc.tile_pool(name="sb1", bufs=1) as sb1, \
             tc.tile_pool(name="lgps", bufs=2, space="PSUM") as lgp, \
             tc.tile_pool(name="grups", bufs=1, space="PSUM") as grup, \
             tc.tile_pool(name="tps", bufs=2, space="PSUM") as tps, \
             tc.tile_pool(name="dr", bufs=2, space="DRAM") as dr:
            # ---------------- load weights ----------------
            wih = wpool.tile([128, KC * 3 * H], F32)          # 4x[128,1536]
            whh = wpool.tile([128, KC * 3 * H], F32)
            wfc = wpool.tile([128, KC * VSP], wfc_dt)         # 4x[128,4096]
            wproj = wpool.tile([128, H], F32)
            zT_sb = wpool.tile([128, B], F32)
            for k in range(KC):
                nc.sync.dma_start(wih[:, k * 3 * H:(k + 1) * 3 * H],
                                  d_wihT[k * 128:(k + 1) * 128, :])
                nc.sync.dma_start(whh[:, k * 3 * H:(k + 1) * 3 * H],
                                  d_whhT[k * 128:(k + 1) * 128, :])
                nc.sync.dma_start(wfc[:, k * VSP:(k + 1) * VSP],
                                  d_wfcT[k * 128:(k + 1) * 128, :])
            nc.sync.dma_start(wproj[:], d_wprojT)
            nc.sync.dma_start(zT_sb[:], d_zT)
            b_gi = wpool.tile([1, 3 * H], F32)
            b_hn = wpool.tile([1, H], F32)
            b_fc = wpool.tile([1, VSP], wfc_dt)
            b_proj = wpool.tile([1, H], F32)
            rank_col = wpool.tile([B, 1], F32)
            nc.sync.dma_start(b_gi[:], d_bias_gi)
            nc.sync.dma_start(b_hn[:], d_bias_hn)
            nc.sync.dma_start(b_fc[:], d_bias_fc)
            nc.sync.dma_start(b_proj[:], d_bias_proj)
            nc.sync.dma_start(rank_col[:], d_rank)
            ident = wpool.tile([B, B], F32)
            make_identity(nc, ident[:])
            ones1 = wpool.tile([1, 128], F32)
            nc.vector.memset(ones1[:], 1.0)

            if logits_f32r:
                ones_r = wpool.tile([1, 128], F32R)
                nc.vector.tensor_copy(ones_r[:], ones1[:])
                cand8 = wpool.tile([B, 8], F32)
                nc.vector.memset(cand8[:], NEG)
                cpair = wpool.tile([B, 8], F32)
                nc.vector.memset(cpair[:], NEG)

            # ---------------- h0 ----------------
            h0_ps = lgp.tile([B, H], F32, tag="lg")
            nc.tensor.matmul(h0_ps[:], zT_sb[:], wproj[:], start=True, stop=False)
            nc.tensor.matmul(h0_ps[:], ones1[0:1, 0:B], b_proj[:],
                             start=False, stop=True)
            h_cur = sb.tile([B, H], F32, tag="h")
            nc.scalar.copy(h_cur[:], h0_ps[:])

            # transposed h (lhsT layout): [128, KC*64], chunk k at [:, 64k:64k+64]
            def transpose_to(dst_sb, src_ap, extra_dst=None):
                tp = tps.tile([128, 256], F32, tag="tp")
                for k in range(KC):
                    nc.tensor.transpose(tp[:, k * 64:(k + 1) * 64],
                                        src_ap[:, k * 128:(k + 1) * 128],
                                        ident[:])
                nc.scalar.copy(dst_sb[:], tp[:])
                if extra_dst is not None:
                    nc.vector.tensor_copy(extra_dst[:], tp[:])

            hT = sb.tile([128, KC * 64], F32, tag="hT")
            if logits_f32r:
                hT_r = sb.tile([128, KC * 64], F32R, tag="hTr")
                transpose_to(hT, h_cur[:], extra_dst=hT_r)
            else:
                hT_r = None
                transpose_to(hT, h_cur[:])

            xT = hT            # step 0: x = h0
            x_sb = None
            ids_i32 = None
            if probe is not None:
                ids_i32 = wpool.tile([B, 1], I32)
                nc.vector.memset(ids_i32[:], 0.0)
                if probe == "gruonly":
                    lg_dummy = wpool.tile([B, VS], F32)
                    nc.vector.memset(lg_dummy[:], 0.0)

            # DRAM bounce tiles for the collective
            for t in range(t_steps):
                # ---------- gh (+ rz biases) : psum ----------
                mtab_step = use_mtab and t > 0
                rz_ps = grup.tile([B, 1024], F32, tag="rz")
                ghn_ps = grup.tile([B, 512], F32, tag="ghn")
                # rz region: gh first (start); gi mms accumulate on top unless
                # this is an mtab step (gi arrives via gather + DVE add).
                for j in range(2):
                    o = rz_ps[:, j * 512:(j + 1) * 512]
                    for k in range(KC):
                        nc.tensor.matmul(o, hT[:, k * 64:(k + 1) * 64],
                                         whh[:, k * 3 * H + j * 512:
                                             k * 3 * H + (j + 1) * 512],
                                         start=(k == 0),
                                         stop=(mtab_step and k == KC - 1))
                # ghn = (h @ W_hh.T)_n + b_hh_n
                for k in range(KC):
                    nc.tensor.matmul(ghn_ps[:], hT[:, k * 64:(k + 1) * 64],
                                     whh[:, k * 3 * H + 1024:k * 3 * H + 1536],
                                     start=(k == 0), stop=False)
                nc.tensor.matmul(ghn_ps[:], ones1[0:1, 0:B], b_hn[:],
                                 start=False, stop=True)

                # ---------- gi ----------
                if mtab_step:
                    # rz: copy gh_rz to SBUF (hidden), then CCE-add gather of
                    # mtab's rz slice lands gi_rz + gh_rz in one DMA.
                    rz_acc = sb.tile([B, 1024], F32, tag="rzacc")
                    nc.scalar.copy(rz_acc[:], rz_ps[:])
                    gin_sb = sb.tile([B, H], F32, tag="ginsb")
                    if probe in ("nogather", "gruonly"):
                        rz_tmp = sb.tile([B, 1024], F32, tag="rztmp")
                        nc.sync.dma_start(rz_tmp[:], d_mtab_rz[0:B, :])
                        nc.vector.tensor_tensor(out=rz_acc[:], in0=rz_acc[:],
                                                in1=rz_tmp[:], op=OP.add)
                        nc.sync.dma_start(gin_sb[:], d_mtab_n[0:B, :])
                    else:
                        nc.gpsimd.indirect_dma_start(
                            out=rz_acc[:], out_offset=None, in_=d_mtab_rz,
                            in_offset=bass.IndirectOffsetOnAxis(ap=ids_i32[:, 0:1], axis=0),
                            compute_op=OP.add)
                        nc.gpsimd.indirect_dma_start(
                            out=gin_sb[:], out_offset=None, in_=d_mtab_n,
                            in_offset=bass.IndirectOffsetOnAxis(ap=ids_i32[:, 0:1], axis=0))
                else:
                    gin_ps = grup.tile([B, 512], F32, tag="gin")
                    for j in range(2):
                        o = rz_ps[:, j * 512:(j + 1) * 512]
                        for k in range(KC):
                            nc.tensor.matmul(o, xT[:, k * 64:(k + 1) * 64],
                                             wih[:, k * 3 * H + j * 512:
                                                 k * 3 * H + (j + 1) * 512],
                                             start=False, stop=False)
                        nc.tensor.matmul(o, ones1[0:1, 0:B],
                                         b_gi[:, j * 512:(j + 1) * 512],
                                         start=False, stop=True)
                    for k in range(KC):
                        nc.tensor.matmul(gin_ps[:], xT[:, k * 64:(k + 1) * 64],
                                         wih[:, k * 3 * H + 1024:k * 3 * H + 1536],
                                         start=(k == 0), stop=False)
                    nc.tensor.matmul(gin_ps[:], ones1[0:1, 0:B],
                                     b_gi[:, 1024:1536], start=False, stop=True)

                # ---------- gates ----------
                rz_sb = sb1.tile([B, 1024], F32, tag="rzsb")
                if mtab_step:
                    nc.scalar.activation(rz_sb[:], rz_acc[:], AF.Sigmoid)
                else:
                    nc.scalar.activation(rz_sb[:], rz_ps[:], AF.Sigmoid)
                u_sb = sb1.tile([B, H], F32, tag="u")
                nc.vector.tensor_tensor(out=u_sb[:], in0=rz_sb[:, 0:512],
                                        in1=ghn_ps[:], op=OP.mult)
                if mtab_step:
                    nc.vector.tensor_tensor(out=u_sb[:], in0=u_sb[:],
                                            in1=gin_sb[:], op=OP.add)
                else:
                    nc.vector.tensor_tensor(out=u_sb[:], in0=u_sb[:],
                                            in1=gin_ps[:], op=OP.add)
                n_sb = sb1.tile([B, H], F32, tag="n")
                nc.scalar.activation(n_sb[:], u_sb[:], AF.Tanh)
                f1_sb = sb1.tile([B, H], F32, tag="f1")
                nc.vector.tensor_tensor(out=f1_sb[:], in0=rz_sb[:, 512:1024],
                                        in1=h_cur[:], op=OP.mult)
                f2_sb = sb1.tile([B, H], F32, tag="f2")
                nc.vector.tensor_scalar(out=f2_sb[:], in0=rz_sb[:, 512:1024],
                                        scalar1=-1.0, scalar2=1.0,
                                        op0=OP.mult, op1=OP.add)
                h_new = sb.tile([B, H], F32, tag="h")
                nc.vector.tensor_tensor(out=h_new[:], in0=f2_sb[:], in1=n_sb[:],
                                        op=OP.mult)
                nc.vector.tensor_tensor(out=h_new[:], in0=h_new[:], in1=f1_sb[:],
                                        op=OP.add)
                h_cur = h_new

                # ---------- hT ----------
                hT = sb.tile([128, KC * 64], F32, tag="hT")
                if logits_f32r:
                    hT_r = sb.tile([128, KC * 64], F32R, tag="hTr")
                    transpose_to(hT, h_cur[:], extra_dst=hT_r)
                else:
                    transpose_to(hT, h_cur[:])

                # ---------- logits ----------
                if probe == "gruonly":
                    nc.sync.dma_start(d_out[:, t * VS:(t + 1) * VS], lg_dummy[:])
                    continue
                lg_sb = sb1.tile([B, VSP], F32, tag="lg_sb")
                tm8 = sb1.tile([B, NVT * 8], F32, tag="tm8")
                lg_lhs = hT_r if logits_f32r else hT
                lg_ones = ones_r if logits_f32r else ones1
                for v in range(NVT):
                    lg_ps = lgp.tile([B, 512], F32, tag="lg")
                    for k in range(KC):
                        nc.tensor.matmul(
                            lg_ps[:], lg_lhs[:, k * 64:(k + 1) * 64],
                            wfc[:, k * VSP + v * 512:k * VSP + (v + 1) * 512],
                            start=(k == 0), stop=False)
                    nc.tensor.matmul(lg_ps[:], lg_ones[0:1, 0:B],
                                     b_fc[:, v * 512:(v + 1) * 512],
                                     start=False, stop=True)
                    nc.scalar.copy(lg_sb[:, v * 512:(v + 1) * 512], lg_ps[:])
                    nc.vector.max(out=tm8[:, v * 8:(v + 1) * 8],
                                  in_=lg_sb[:, v * 512:(v + 1) * 512])

                # DMA logits out (write only the real 4000)
                nc.sync.dma_start(d_out[:, t * VS:(t + 1) * VS], lg_sb[:, 0:VS])

                if t == t_steps - 1:
                    break       # no feedback needed after last step

                if probe is not None:
                    continue    # ids stay 0; skip argmax + exchange

                # ---------- local argmax ----------
                gmax = sb1.tile([B, 1], F32, tag="gmax")
                nc.vector.tensor_reduce(out=gmax[:], in_=tm8[:], axis=AX.X,
                                        op=OP.max)
                mi8 = sb1.tile([B, 8], U32, tag="mi8")
                if logits_f32r:
                    # fp32r logits are ~1.7e-4 loose; find top-3 candidates
                    # from the approximate logits, then re-evaluate them
                    # exactly in fp32 to pick the true argmax.
                    nc.vector.tensor_copy(cand8[:, 0:1], gmax[:])
                    v2t = sb1.tile([B, NVT * 8], F32, tag="v2t")
                    nc.vector.match_replace(out=v2t[:], in_to_replace=cand8[:, 0:8],
                                            in_values=tm8[:], imm_value=NEG)
                    v2v = sb1.tile([B, 1], F32, tag="v2v")
                    nc.vector.tensor_reduce(out=v2v[:], in_=v2t[:], axis=AX.X,
                                            op=OP.max)
                    nc.vector.tensor_copy(cpair[:, 0:1], v2v[:])
                    v3t = sb1.tile([B, NVT * 8], F32, tag="v3t")
                    nc.vector.match_replace(out=v3t[:], in_to_replace=cpair[:, 0:8],
                                            in_values=v2t[:], imm_value=NEG)
                    v3v = sb1.tile([B, 1], F32, tag="v3v")
                    nc.vector.tensor_reduce(out=v3v[:], in_=v3t[:], axis=AX.X,
                                            op=OP.max)
                    cand3 = sb1.tile([B, 8], F32, tag="cand3")
                    nc.vector.memset(cand3[:], NEG)
                    nc.vector.tensor_copy(cand3[:, 0:1], gmax[:])
                    nc.vector.tensor_copy(cand3[:, 1:2], v2v[:])
                    nc.vector.tensor_copy(cand3[:, 2:3], v3v[:])
                    nc.vector.max_index(out=mi8[:], in_max=cand3[:],
                                        in_values=lg_sb[:])
                    idl = sb1.tile([B, 8], I32, tag="idl")
                    nc.vector.tensor_copy(idl[:, 0:3], mi8[:, 0:3])
                    wb3 = sb1.tile([B, 3 * (E + 1)], F32, tag="wb3")
                    p3 = sb1.tile([B, 3 * E], F32, tag="p3")
                    for j in range(3):
                        nc.gpsimd.indirect_dma_start(
                            out=wb3[:, j * (E + 1):(j + 1) * (E + 1)],
                            out_offset=None, in_=d_wb,
                            in_offset=bass.IndirectOffsetOnAxis(
                                ap=idl[:, j:j + 1], axis=0))
                        nc.vector.tensor_tensor(
                            out=p3[:, j * E:(j + 1) * E], in0=h_cur[:],
                            in1=wb3[:, j * (E + 1):j * (E + 1) + E], op=OP.mult)
                    e3 = sb1.tile([B, 4], F32, tag="e3")
                    nc.vector.tensor_reduce(
                        out=e3[:, 0:3],
                        in_=p3[:].rearrange("p (j e) -> p j e", j=3),
                        axis=AX.X, op=OP.add)
                    # add per-candidate bias (wb3 col E of each 513-block)
                    nc.vector.tensor_tensor(
                        out=e3[:, 0:3], in0=e3[:, 0:3],
                        in1=wb3[:].rearrange("p (j e) -> p j e", j=3)[:, :, E:E + 1].squeeze(),
                        op=OP.add)
                    idf = sb1.tile([B, 8], F32, tag="idf")
                    nc.vector.tensor_copy(idf[:, 0:3], mi8[:, 0:3])
                    nc.vector.tensor_scalar(out=idf[:, 0:3], in0=idf[:, 0:3],
                                            scalar1=rank_col[:, 0:1],
                                            scalar2=None, op0=OP.add)
                    cmp01 = sb1.tile([B, 1], I32, tag="cmp01")
                    nc.vector.tensor_tensor(out=cmp01[:], in0=e3[:, 1:2],
                                            in1=e3[:, 0:1], op=OP.is_gt)
                    m01 = sb1.tile([B, 1], F32, tag="m01")
                    nc.vector.tensor_tensor(out=m01[:], in0=e3[:, 0:1],
                                            in1=e3[:, 1:2], op=OP.max)
                    g01 = sb1.tile([B, 1], F32, tag="g01")
                    nc.vector.select(out=g01[:], mask=cmp01[:],
                                     on_true=idf[:, 1:2], on_false=idf[:, 0:1])
                    cmp2 = sb1.tile([B, 1], I32, tag="cmp2")
                    nc.vector.tensor_tensor(out=cmp2[:], in0=e3[:, 2:3],
                                            in1=m01[:], op=OP.is_gt)
                    emax = sb1.tile([B, 1], F32, tag="emaxv")
                    nc.vector.tensor_tensor(out=emax[:], in0=m01[:],
                                            in1=e3[:, 2:3], op=OP.max)
                    gidf = sb1.tile([B, 1], F32, tag="gidf")
                    nc.vector.select(out=gidf[:], mask=cmp2[:],
                                     on_true=idf[:, 2:3], on_false=g01[:])
                else:
                    gmax8 = sb1.tile([B, 8], F32, tag="gmax8")
                    nc.vector.tensor_copy(gmax8[:], gmax[:].to_broadcast([B, 8]))
                    nc.vector.max_index(out=mi8[:], in_max=gmax8[:],
                                        in_values=lg_sb[:])
                    emax = gmax
                    gidf = sb1.tile([B, 1], F32, tag="gidf")
                    nc.vector.tensor_copy(gidf[:], mi8[:, 0:1])
                    nc.vector.tensor_scalar(out=gidf[:], in0=gidf[:],
                                            scalar1=rank_col[:, 0:1], scalar2=None,
                                            op0=OP.add)

                # ---------- AllGather of (emax, gid) ----------
                pay = sb1.tile([B, 2], F32, tag="pay")
                nc.vector.tensor_copy(pay[:, 0:1], emax[:])
                nc.vector.tensor_copy(pay[:, 1:2], gidf[:])
                payT_ps = tps.tile([128, 256], F32, tag="tp")
                nc.tensor.transpose(payT_ps[0:1, 0:64], pay[:, 0:1], ident[:])
                nc.tensor.transpose(payT_ps[0:1, 64:128], pay[:, 1:2], ident[:])
                pay_row = sb1.tile([1, 128], F32, tag="payrow")
                nc.vector.tensor_copy(pay_row[:], payT_ps[0:1, 0:128])
                cc_in = dr.tile([1, 128], F32, tag="ccin")
                cc_out = dr.tile([NC_N, 128], F32, tag="ccout")
                nc.gpsimd.dma_start(cc_in[:], pay_row[:])
                if no_cc:
                    # perf probe only (wrong results): skip the collective,
                    # replicate own payload into all 8 rows
                    for rr in range(NC_N):
                        nc.gpsimd.dma_start(cc_out[rr:rr + 1, :], cc_in[:])
                else:
                    nc.gpsimd.collective_compute(
                        "AllGather", OP.bypass,
                        replica_groups=[list(range(NC_N))],
                        ins=[cc_in[:].opt()], outs=[cc_out[:].opt()])
                ag_sb = sb1.tile([NC_N, 128], F32, tag="agsb")
                nc.gpsimd.dma_start(ag_sb[:], cc_out[:])

                # ---------- global argmax ----------
                agT_ps = tps.tile([128, 256], F32, tag="tp")
                nc.tensor.transpose(agT_ps[0:B, 0:8], ag_sb[:, 0:64],
                                    ident[0:8, 0:8])
                nc.tensor.transpose(agT_ps[0:B, 8:16], ag_sb[:, 64:128],
                                    ident[0:8, 0:8])
                agT = sb1.tile([B, 16], F32, tag="agTsb")
                nc.vector.tensor_copy(agT[:], agT_ps[0:B, 0:16])
                gm = sb1.tile([B, 1], F32, tag="gm")
                nc.vector.tensor_reduce(out=gm[:], in_=agT[:, 0:8], axis=AX.X,
                                        op=OP.max)
                mask = sb1.tile([B, 8], F32, tag="mask")
                nc.vector.tensor_scalar(out=mask[:], in0=agT[:, 0:8],
                                        scalar1=gm[:, 0:1], scalar2=None,
                                        op0=OP.is_lt)    # 1.0 where NOT max
                sel = sb1.tile([B, 8], F32, tag="sel")
                # sel = gid + (not-max)*BIG  -> min over ranks = winning gid
                nc.vector.tensor_scalar(out=sel[:], in0=mask[:],
                                        scalar1=2.0e9, scalar2=None,
                                        op0=OP.mult)
                nc.vector.tensor_tensor(out=sel[:], in0=sel[:], in1=agT[:, 8:16],
                                        op=OP.add)
                widf = sb1.tile([B, 1], F32, tag="widf")
                nc.vector.tensor_reduce(out=widf[:], in_=sel[:], axis=AX.X,
                                        op=OP.min)
                ids_i32 = sb1.tile([B, 1], I32, tag="ids")
                nc.vector.tensor_copy(ids_i32[:], widf[:])

                # ---------- feedback gather + xT ----------
                if not use_mtab:
                    x_sb = sb.tile([B, E], F32, tag="x")
                    nc.gpsimd.indirect_dma_start(
                        out=x_sb[:], out_offset=None, in_=d_emb,
                        in_offset=bass.IndirectOffsetOnAxis(ap=ids_i32[:, 0:1],
                                                            axis=0))
                    xT = sb.tile([128, KC * 64], F32, tag="xT")
                    transpose_to(xT, x_sb[:])

    nc.compile()
    return nc


_BUILT = {}


def _get_nc():
    key = (T, LOGITS_F32R, USE_MTAB)
    if key not in _BUILT:
        _BUILT[key] = build(T, LOGITS_F32R, USE_MTAB)
    return _BUILT[key]


def make_in_maps(z, emb, W_proj, b_proj, W_ih, b_ih, W_hh, b_hh, W_fc, b_fc):
    z = np.asarray(z, np.float32)
    emb = np.ascontiguousarray(np.asarray(emb, np.float32))
    W_proj = np.asarray(W_proj, np.float32)
    W_ih = np.asarray(W_ih, np.float32)
    W_hh = np.asarray(W_hh, np.float32)
    W_fc = np.asarray(W_fc, np.float32)
    b_proj = np.asarray(b_proj, np.float32)
    b_ih = np.asarray(b_ih, np.float32)
    b_hh = np.asarray(b_hh, np.float32)
    b_fc = np.asarray(b_fc, np.float32)

    wihT = np.ascontiguousarray(W_ih.T)            # [512, 1536]
    whhT = np.ascontiguousarray(W_hh.T)
    wprojT = np.ascontiguousarray(W_proj.T)        # [128, 512]
    zT = np.ascontiguousarray(z.T)                 # [128, 64]
    bias_gi = b_ih.copy()
    bias_gi[0:1024] += b_hh[0:1024]
    bias_gi = bias_gi[None, :]                     # [1, 1536]
    bias_hn = b_hh[None, 1024:1536]
    bias_proj = b_proj[None, :]

    common = dict(wihT=wihT, whhT=whhT, wprojT=wprojT, zT=zT,
                  bias_gi=bias_gi, bias_hn=bias_hn, bias_proj=bias_proj)
    if USE_MTAB:
        mtab = (emb @ W_ih.T + bias_gi).astype(np.float32)
        common["mtab_rz"] = np.ascontiguousarray(mtab[:, 0:1024])
        common["mtab_n"] = np.ascontiguousarray(mtab[:, 1024:1536])
    else:
        common["emb"] = emb

    in_maps = []
    for c in range(NC_N):
        wfc_sh = W_fc[c * VS:(c + 1) * VS, :]          # [4000, 512]
        wfcT = np.zeros((H, VSP), np.float32)
        wfcT[:, 0:VS] = wfc_sh.T
        bias_fc = np.full((1, VSP), NEG, np.float32)
        bias_fc[0, 0:VS] = b_fc[c * VS:(c + 1) * VS]
        rank_col = np.full((B, 1), float(c * VS), np.float32)
        m = dict(common)
        m.update(wfcT=np.ascontiguousarray(wfcT), bias_fc=bias_fc,
                 rank_col=rank_col)
        if LOGITS_F32R:
            m["wb"] = np.ascontiguousarray(
                np.concatenate([wfc_sh, b_fc[c * VS:(c + 1) * VS, None]],
                               axis=1))
        in_maps.append(m)
    return in_maps


def kernel(z, emb, W_proj, b_proj, W_ih, b_ih, W_hh, b_hh, W_fc, b_fc,
           context_length):
    assert int(context_length) == T
    nc = _get_nc()
    in_maps = make_in_maps(z, emb, W_proj, b_proj, W_ih, b_ih, W_hh, b_hh,
                           W_fc, b_fc)
    res = bass_utils.run_bass_kernel_spmd(nc, in_maps,
                                          core_ids=list(range(NC_N)))
    shards = [res.results[c]["out"].reshape(B, T, VS) for c in range(NC_N)]
    return np.concatenate(shards, axis=2)



# revision 21
# speedup vs baseline: 8.3201x; 1.9111x over previous
"""Trainium2 Bass kernel for nn_AutoregressiveDecoder (GRU decoder w/ greedy argmax feedback).

B=64, L=128, E=512, H=512, V=32000, T=64, 8 NeuronCores.

Sharding: vocab (V) split 8 ways; each core holds its W_fc.T shard resident in
SBUF, computes the full GRU (replicated) in fp32, its logits shard, and its
local (max, argmax).  A per-step AllGather of the 8 (max, argmax) pairs gives
every core the global argmax; feedback x = emb[ids] comes from an indirect DMA
gather out of a full emb copy in each core's DRAM.

Self-contained: hardcodes shapes; only imports the platform toolchain.
"""
import sys

if "/opt/trn_rl_repo" not in sys.path:
    sys.path.insert(0, "/opt/trn_rl_repo")

import numpy as np

import concourse.bass as bass
import concourse.mybir as mybir
import concourse.bacc as bacc
import concourse.tile as tile
import concourse.bass_utils as bass_utils
from concourse.masks import make_identity

F32 = mybir.dt.float32
F32R = mybir.dt.float32r
U32 = mybir.dt.uint32
I32 = mybir.dt.int32
AF = mybir.ActivationFunctionType
OP = mybir.AluOpType
AX = mybir.AxisListType

B, L, E, H, V, T = 64, 128, 512, 512, 32000, 64
NC_N = 8
VS = V // NC_N          # 4000 vocab per core
VSP = 4096              # padded (8 tiles of 512)
KC = H // 128           # 4 contraction chunks
NVT = VSP // 512        # 8 vocab tiles per core
NEG = -1.0e30

# ---- build flags ----
LOGITS_F32R = True      # fast fp32r logits + exact fp32 top-3 re-eval
USE_MTAB = True         # gi via gather from host-precomputed emb @ W_ih.T


def _mm_acc(nc, out_ap, lhsT, rhs_list, start_first):
    """Accumulating matmul helper: sequence of (lhsT_ap, rhs_ap) into out."""
    n = len(rhs_list)
    for i, (lt, rh) in enumerate(rhs_list):
        nc.tensor.matmul(out_ap, lt, rh,
                         start=(start_first and i == 0), stop=(i == n - 1))


def build(t_steps=T, logits_f32r=LOGITS_F32R, use_mtab=USE_MTAB, no_cc=False,
          probe=None):
    # probe (perf-only, wrong results):
    #   "noargmax": skip local/global argmax + exchange; ids forced to 0
    #   "nogather": noargmax + mtab indirect gathers -> regular row-0 DMAs
    #   "gruonly":  nogather + skip logits matmuls (dummy output writes)
    nc = bacc.Bacc("TRN2", target_bir_lowering=False, debug=False,
                   num_devices=NC_N)

    # ---------------- DRAM I/O ----------------
    d_emb = None
    if not use_mtab:
        d_emb = nc.dram_tensor("emb", [V, E], F32, kind="ExternalInput").ap()
    d_wihT = nc.dram_tensor("wihT", [H, 3 * H], F32, kind="ExternalInput").ap()
    d_whhT = nc.dram_tensor("whhT", [H, 3 * H], F32, kind="ExternalInput").ap()
    wfc_dt = F32R if logits_f32r else F32
    d_wfcT = nc.dram_tensor("wfcT", [H, VSP], wfc_dt, kind="ExternalInput").ap()
    d_wprojT = nc.dram_tensor("wprojT", [L, H], F32, kind="ExternalInput").ap()
    d_zT = nc.dram_tensor("zT", [L, B], F32, kind="ExternalInput").ap()
    d_bias_gi = nc.dram_tensor("bias_gi", [1, 3 * H], F32, kind="ExternalInput").ap()
    d_bias_hn = nc.dram_tensor("bias_hn", [1, H], F32, kind="ExternalInput").ap()
    d_bias_fc = nc.dram_tensor("bias_fc", [1, VSP], wfc_dt, kind="ExternalInput").ap()
    d_bias_proj = nc.dram_tensor("bias_proj", [1, H], F32, kind="ExternalInput").ap()
    d_rank = nc.dram_tensor("rank_col", [B, 1], F32, kind="ExternalInput").ap()
    d_rmask = nc.dram_tensor("rank_mask16", [B, 2 * NC_N], F32,
                             kind="ExternalInput").ap()
    if use_mtab:
        d_mtab = nc.dram_tensor("mtab", [V, 3 * H], F32, kind="ExternalInput").ap()
    if logits_f32r:
        # per-core shard of [W_fc | b_fc] for exact candidate re-evaluation
        d_wb = nc.dram_tensor("wb", [VS, E + 1], F32, kind="ExternalInput").ap()
    d_out = nc.dram_tensor("out", [B, t_steps * VS], F32, kind="ExternalOutput").ap()

    with tile.TileContext(nc) as tc:
        with tc.tile_pool(name="wts", bufs=1) as wpool, \
             tc.tile_pool(name="sb", bufs=2) as sb, \
             tc.tile_pool(name="sb1", bufs=1) as sb1, \
             tc.tile_pool(name="lgps", bufs=2, space="PSUM") as lgp, \
             tc.tile_pool(name="grups", bufs=1, space="PSUM") as grup, \
             tc.tile_pool(name="tps", bufs=2, space="PSUM") as tps, \
             tc.tile_pool(name="dr", bufs=2, space="DRAM") as dr:
            # ---------------- load weights ----------------
            wih = wpool.tile([128, KC * 3 * H], F32)          # 4x[128,1536]
            whh = wpool.tile([128, KC * 3 * H], F32)
            wfc = wpool.tile([128, KC * VSP], wfc_dt)         # 4x[128,4096]
            wproj = wpool.tile([128, H], F32)
            zT_sb = wpool.tile([128, B], F32)
            for k in range(KC):
                nc.sync.dma_start(wih[:, k * 3 * H:(k + 1) * 3 * H],
                                  d_wihT[k * 128:(k + 1) * 128, :])
                nc.sync.dma_start(whh[:, k * 3 * H:(k + 1) * 3 * H],
                                  d_whhT[k * 128:(k + 1) * 128, :])
                nc.sync.dma_start(wfc[:, k * VSP:(k + 1) * VSP],
                                  d_wfcT[k * 128:(k + 1) * 128, :])
            nc.sync.dma_start(wproj[:], d_wprojT)
            nc.sync.dma_start(zT_sb[:], d_zT)
            b_gi = wpool.tile([1, 3 * H], F32)
            b_hn = wpool.tile([1, H], F32)
            b_fc = wpool.tile([1, VSP], wfc_dt)
            b_proj = wpool.tile([1, H], F32)
            rank_col = wpool.tile([B, 1], F32)
            rmask16 = wpool.tile([B, 2 * NC_N], F32)
            nc.sync.dma_start(b_gi[:], d_bias_gi)
            nc.sync.dma_start(b_hn[:], d_bias_hn)
            nc.sync.dma_start(b_fc[:], d_bias_fc)
            nc.sync.dma_start(b_proj[:], d_bias_proj)
            nc.sync.dma_start(rank_col[:], d_rank)
            nc.sync.dma_start(rmask16[:], d_rmask)
            ident = wpool.tile([B, B], F32)
            make_identity(nc, ident[:])
            ones1 = wpool.tile([1, 128], F32)
            nc.vector.memset(ones1[:], 1.0)

            if logits_f32r:
                ones_r = wpool.tile([1, 128], F32R)
                nc.vector.tensor_copy(ones_r[:], ones1[:])
                cand8 = wpool.tile([B, 8], F32)
                nc.vector.memset(cand8[:], NEG)
                cpair = wpool.tile([B, 8], F32)
                nc.vector.memset(cpair[:], NEG)
                cand3 = wpool.tile([B, 8], F32)
                nc.vector.memset(cand3[:], NEG)

            # ---------------- h0 ----------------
            h0_ps = lgp.tile([B, H], F32, tag="lg")
            nc.tensor.matmul(h0_ps[:], zT_sb[:], wproj[:], start=True, stop=False)
            nc.tensor.matmul(h0_ps[:], ones1[0:1, 0:B], b_proj[:],
                             start=False, stop=True)
            h_cur = sb.tile([B, H], F32, tag="h")
            nc.scalar.copy(h_cur[:], h0_ps[:])

            # transposed h (lhsT layout): [128, KC*64], chunk k at [:, 64k:64k+64]
            def transpose_to(dst_sb, src_ap, extra_dst=None):
                tp = tps.tile([128, 256], F32, tag="tp")
                for k in range(KC):
                    nc.tensor.transpose(tp[:, k * 64:(k + 1) * 64],
                                        src_ap[:, k * 128:(k + 1) * 128],
                                        ident[:])
                nc.scalar.copy(dst_sb[:], tp[:])
                if extra_dst is not None:
                    nc.vector.tensor_copy(extra_dst[:], tp[:])

            hT = sb.tile([128, KC * 64], F32, tag="hT")
            if logits_f32r:
                hT_r = sb.tile([128, KC * 64], F32R, tag="hTr")
                transpose_to(hT, h_cur[:], extra_dst=hT_r)
            else:
                hT_r = None
                transpose_to(hT, h_cur[:])

            xT = hT            # step 0: x = h0
            x_sb = None
            ids_i32 = None
            if probe is not None:
                ids_i32 = wpool.tile([B, 1], I32)
                nc.vector.memset(ids_i32[:], 0.0)
                if probe == "gruonly":
                    lg_dummy = wpool.tile([B, VS], F32)
                    nc.vector.memset(lg_dummy[:], 0.0)

            # DRAM bounce tiles for the collective
            for t in range(t_steps):
                # ---------- gh (+ rz biases) : psum ----------
                mtab_step = use_mtab and t > 0
                rz_ps = grup.tile([B, 1024], F32, tag="rz")
                ghn_ps = grup.tile([B, 512], F32, tag="ghn")
                # rz region: gh first (start); gi mms accumulate on top unless
                # this is an mtab step (gi arrives via gather + DVE add).
                for j in range(2):
                    o = rz_ps[:, j * 512:(j + 1) * 512]
                    for k in range(KC):
                        nc.tensor.matmul(o, hT[:, k * 64:(k + 1) * 64],
                                         whh[:, k * 3 * H + j * 512:
                                             k * 3 * H + (j + 1) * 512],
                                         start=(k == 0),
                                         stop=(mtab_step and k == KC - 1))
                # ghn = (h @ W_hh.T)_n + b_hh_n
                for k in range(KC):
                    nc.tensor.matmul(ghn_ps[:], hT[:, k * 64:(k + 1) * 64],
                                     whh[:, k * 3 * H + 1024:k * 3 * H + 1536],
                                     start=(k == 0), stop=False)
                nc.tensor.matmul(ghn_ps[:], ones1[0:1, 0:B], b_hn[:],
                                 start=False, stop=True)

                # ---------- gi ----------
                if mtab_step:
                    # one gather of the full mtab row [B, 1536] = gi (+b_gi),
                    # issued as soon as ids are known (overlaps gh matmuls);
                    # the rz half is added to gh_rz on DVE afterwards.
                    gall = sb1.tile([B, 3 * H], F32, tag="gall")
                    if probe in ("nogather", "gruonly"):
                        nc.sync.dma_start(gall[:], d_mtab[0:B, :])
                    else:
                        nc.gpsimd.indirect_dma_start(
                            out=gall[:], out_offset=None, in_=d_mtab,
                            in_offset=bass.IndirectOffsetOnAxis(ap=ids_i32[:, 0:1], axis=0))
                    rz_acc = sb1.tile([B, 1024], F32, tag="rzacc")
                    nc.vector.tensor_tensor(out=rz_acc[:], in0=gall[:, 0:1024],
                                            in1=rz_ps[:], op=OP.add)
                    gin_sb = gall[:, 1024:1536]
                else:
                    gin_ps = grup.tile([B, 512], F32, tag="gin")
                    for j in range(2):
                        o = rz_ps[:, j * 512:(j + 1) * 512]
                        for k in range(KC):
                            nc.tensor.matmul(o, xT[:, k * 64:(k + 1) * 64],
                                             wih[:, k * 3 * H + j * 512:
                                                 k * 3 * H + (j + 1) * 512],
                                             start=False, stop=False)
                        nc.tensor.matmul(o, ones1[0:1, 0:B],
                                         b_gi[:, j * 512:(j + 1) * 512],
                                         start=False, stop=True)
                    for k in range(KC):
                        nc.tensor.matmul(gin_ps[:], xT[:, k * 64:(k + 1) * 64],
                                         wih[:, k * 3 * H + 1024:k * 3 * H + 1536],
                                         start=(k == 0), stop=False)
                    nc.tensor.matmul(gin_ps[:], ones1[0:1, 0:B],
                                     b_gi[:, 1024:1536], start=False, stop=True)

                # ---------- gates ----------
                rz_sb = sb1.tile([B, 1024], F32, tag="rzsb")
                if mtab_step:
                    nc.scalar.activation(rz_sb[:], rz_acc[:], AF.Sigmoid)
                else:
                    nc.scalar.activation(rz_sb[:], rz_ps[:], AF.Sigmoid)
                u_sb = sb1.tile([B, H], F32, tag="u")
                nc.vector.tensor_tensor(out=u_sb[:], in0=rz_sb[:, 0:512],
                                        in1=ghn_ps[:], op=OP.mult)
                if mtab_step:
                    nc.vector.tensor_tensor(out=u_sb[:], in0=u_sb[:],
                                            in1=gall[:, 1024:1536], op=OP.add)
                else:
                    nc.vector.tensor_tensor(out=u_sb[:], in0=u_sb[:],
                                            in1=gin_ps[:], op=OP.add)
                n_sb = sb1.tile([B, H], F32, tag="n")
                nc.scalar.activation(n_sb[:], u_sb[:], AF.Tanh)
                # h_new = n + z*(h - n)
                hd_sb = sb1.tile([B, H], F32, tag="hd")
                nc.vector.tensor_tensor(out=hd_sb[:], in0=h_cur[:], in1=n_sb[:],
                                        op=OP.subtract)
                h_new = sb.tile([B, H], F32, tag="h")
                nc.vector.tensor_tensor(out=h_new[:], in0=rz_sb[:, 512:1024],
                                        in1=hd_sb[:], op=OP.mult)
                nc.vector.tensor_tensor(out=h_new[:], in0=h_new[:], in1=n_sb[:],
                                        op=OP.add)
                h_cur = h_new

                # ---------- hT ----------
                hT = sb.tile([128, KC * 64], F32, tag="hT")
                if logits_f32r:
                    hT_r = sb.tile([128, KC * 64], F32R, tag="hTr")
                    transpose_to(hT, h_cur[:], extra_dst=hT_r)
                else:
                    transpose_to(hT, h_cur[:])

                # ---------- logits ----------
                if probe == "gruonly":
                    nc.sync.dma_start(d_out[:, t * VS:(t + 1) * VS], lg_dummy[:])
                    continue
                lg_sb = sb1.tile([B, VSP], F32, tag="lg_sb")
                tm8 = sb1.tile([B, NVT * 8], F32, tag="tm8")
                lg_lhs = hT_r if logits_f32r else hT
                lg_ones = ones_r if logits_f32r else ones1
                for v in range(NVT):
                    lg_ps = lgp.tile([B, 512], F32, tag="lg")
                    for k in range(KC):
                        nc.tensor.matmul(
                            lg_ps[:], lg_lhs[:, k * 64:(k + 1) * 64],
                            wfc[:, k * VSP + v * 512:k * VSP + (v + 1) * 512],
                            start=(k == 0), stop=False)
                    nc.tensor.matmul(lg_ps[:], lg_ones[0:1, 0:B],
                                     b_fc[:, v * 512:(v + 1) * 512],
                                     start=False, stop=True)
                    nc.scalar.copy(lg_sb[:, v * 512:(v + 1) * 512], lg_ps[:])
                    nc.vector.max(out=tm8[:, v * 8:(v + 1) * 8],
                                  in_=lg_sb[:, v * 512:(v + 1) * 512])

                # DMA logits out (write only the real 4000)
                nc.sync.dma_start(d_out[:, t * VS:(t + 1) * VS], lg_sb[:, 0:VS])

                if t == t_steps - 1:
                    break       # no feedback needed after last step

                if probe is not None:
                    continue    # ids stay 0; skip argmax + exchange

                # ---------- local argmax ----------
                gmax = sb1.tile([B, 1], F32, tag="gmax")
                nc.vector.tensor_reduce(out=gmax[:], in_=tm8[:], axis=AX.X,
                                        op=OP.max)
                mi8 = sb1.tile([B, 8], U32, tag="mi8")
                if logits_f32r:
                    # fp32r logits are ~1.7e-4 loose; find top-3 candidates
                    # from the approximate logits, then re-evaluate them
                    # exactly in fp32 to pick the true argmax.
                    nc.vector.tensor_copy(cand8[:, 0:1], gmax[:])
                    v2t = sb1.tile([B, NVT * 8], F32, tag="v2t")
                    nc.vector.match_replace(out=v2t[:], in_to_replace=cand8[:, 0:8],
                                            in_values=tm8[:], imm_value=NEG)
                    v2v = sb1.tile([B, 1], F32, tag="v2v")
                    nc.vector.tensor_reduce(out=v2v[:], in_=v2t[:], axis=AX.X,
                                            op=OP.max)
                    nc.vector.tensor_copy(cpair[:, 0:1], v2v[:])
                    v3t = sb1.tile([B, NVT * 8], F32, tag="v3t")
                    nc.vector.match_replace(out=v3t[:], in_to_replace=cpair[:, 0:8],
                                            in_values=v2t[:], imm_value=NEG)
                    v3v = sb1.tile([B, 1], F32, tag="v3v")
                    nc.vector.tensor_reduce(out=v3v[:], in_=v3t[:], axis=AX.X,
                                            op=OP.max)
                    nc.vector.tensor_copy(cand3[:, 0:1], gmax[:])
                    nc.vector.tensor_copy(cand3[:, 1:2], v2v[:])
                    nc.vector.tensor_copy(cand3[:, 2:3], v3v[:])
                    nc.vector.max_index(out=mi8[:], in_max=cand3[:],
                                        in_values=lg_sb[:])
                    idl = sb1.tile([B, 8], I32, tag="idl")
                    nc.vector.tensor_copy(idl[:, 0:3], mi8[:, 0:3])
                    wb3 = sb1.tile([B, 3 * (E + 1)], F32, tag="wb3")
                    p3 = sb1.tile([B, 3 * E], F32, tag="p3")
                    nc.gpsimd.indirect_dma_start(
                        out=wb3[:], out_offset=None, in_=d_wb,
                        in_offset=bass.IndirectOffsetOnAxis(
                            ap=idl[:, 0:3], axis=0))
                    for j in range(3):
                        nc.vector.tensor_tensor(
                            out=p3[:, j * E:(j + 1) * E], in0=h_cur[:],
                            in1=wb3[:, j * (E + 1):j * (E + 1) + E], op=OP.mult)
                    e3 = sb1.tile([B, 4], F32, tag="e3")
                    nc.vector.tensor_reduce(
                        out=e3[:, 0:3],
                        in_=p3[:].rearrange("p (j e) -> p j e", j=3),
                        axis=AX.X, op=OP.add)
                    # add per-candidate bias (wb3 col E of each 513-block)
                    nc.vector.tensor_tensor(
                        out=e3[:, 0:3], in0=e3[:, 0:3],
                        in1=wb3[:].rearrange("p (j e) -> p j e", j=3)[:, :, E:E + 1].squeeze(),
                        op=OP.add)
                    idf = sb1.tile([B, 8], F32, tag="idf")
                    nc.vector.tensor_copy(idf[:, 0:3], mi8[:, 0:3])
                    nc.vector.tensor_scalar(out=idf[:, 0:3], in0=idf[:, 0:3],
                                            scalar1=rank_col[:, 0:1],
                                            scalar2=None, op0=OP.add)
                    cmp01 = sb1.tile([B, 1], I32, tag="cmp01")
                    nc.vector.tensor_tensor(out=cmp01[:], in0=e3[:, 1:2],
                                            in1=e3[:, 0:1], op=OP.is_gt)
                    m01 = sb1.tile([B, 1], F32, tag="m01")
                    nc.vector.tensor_tensor(out=m01[:], in0=e3[:, 0:1],
                                            in1=e3[:, 1:2], op=OP.max)
                    g01 = sb1.tile([B, 1], F32, tag="g01")
                    nc.vector.select(out=g01[:], mask=cmp01[:],
                                     on_true=idf[:, 1:2], on_false=idf[:, 0:1])
                    cmp2 = sb1.tile([B, 1], I32, tag="cmp2")
                    nc.vector.tensor_tensor(out=cmp2[:], in0=e3[:, 2:3],
                                            in1=m01[:], op=OP.is_gt)
                    pay2 = sb1.tile([B, 2], F32, tag="pay2")
                    nc.vector.tensor_tensor(out=pay2[:, 0:1], in0=m01[:],
                                            in1=e3[:, 2:3], op=OP.max)
                    nc.vector.select(out=pay2[:, 1:2], mask=cmp2[:],
                                     on_true=idf[:, 2:3], on_false=g01[:])
                else:
                    gmax8 = sb1.tile([B, 8], F32, tag="gmax8")
                    nc.vector.tensor_copy(gmax8[:], gmax[:].to_broadcast([B, 8]))
                    nc.vector.max_index(out=mi8[:], in_max=gmax8[:],
                                        in_values=lg_sb[:])
                    pay2 = sb1.tile([B, 2], F32, tag="pay2")
                    nc.vector.tensor_copy(pay2[:, 0:1], gmax[:])
                    nc.vector.tensor_copy(pay2[:, 1:2], mi8[:, 0:1])
                    nc.vector.tensor_scalar(out=pay2[:, 1:2], in0=pay2[:, 1:2],
                                            scalar1=rank_col[:, 0:1], scalar2=None,
                                            op0=OP.add)

                # ---------- masked AllReduce(add) of (emax, gid) pairs ----------
                # core r contributes its pair only in cols (2r, 2r+1); the sum
                # assembles the full 8-core table on every core, no transposes.
                cc16 = sb1.tile([B, 2 * NC_N], F32, tag="cc16")
                nc.vector.tensor_tensor(
                    out=cc16[:].rearrange("p (r t) -> p r t", r=NC_N),
                    in0=pay2[:].rearrange("p (o t) -> p o t", o=1)
                        .to_broadcast([B, NC_N, 2]),
                    in1=rmask16[:].rearrange("p (r t) -> p r t", r=NC_N),
                    op=OP.mult)
                cc_in = dr.tile([B, 2 * NC_N], F32, tag="ccin")
                cc_out = dr.tile([B, 2 * NC_N], F32, tag="ccout")
                nc.sync.dma_start(cc_in[:], cc16[:])
                if no_cc:
                    # perf probe only (wrong results): skip the collective
                    nc.sync.dma_start(cc_out[:], cc_in[:])
                else:
                    nc.gpsimd.collective_compute(
                        "AllReduce", OP.add,
                        replica_groups=[list(range(NC_N))],
                        ins=[cc_in[:].opt()], outs=[cc_out[:].opt()])
                ag16 = sb1.tile([B, 2 * NC_N], F32, tag="agsb")
                nc.sync.dma_start(ag16[:], cc_out[:])

                # ---------- global argmax ----------
                vag = ag16[:].rearrange("p (r t) -> p r t", r=NC_N)
                gm = sb1.tile([B, 1], F32, tag="gm")
                nc.vector.tensor_reduce(out=gm[:], in_=vag[:, :, 0:1].squeeze(),
                                        axis=AX.X, op=OP.max)
                mask = sb1.tile([B, 8], F32, tag="mask")
                nc.vector.tensor_scalar(out=mask[:], in0=vag[:, :, 0:1].squeeze(),
                                        scalar1=gm[:, 0:1], scalar2=None,
                                        op0=OP.is_lt)    # 1.0 where NOT max
                sel = sb1.tile([B, 8], F32, tag="sel")
                # sel = gid + (not-max)*BIG  -> min over ranks = winning gid
                nc.vector.tensor_scalar(out=sel[:], in0=mask[:],
                                        scalar1=2.0e9, scalar2=None,
                                        op0=OP.mult)
                nc.vector.tensor_tensor(out=sel[:], in0=sel[:],
                                        in1=vag[:, :, 1:2].squeeze(), op=OP.add)
                widf = sb1.tile([B, 1], F32, tag="widf")
                nc.vector.tensor_reduce(out=widf[:], in_=sel[:], axis=AX.X,
                                        op=OP.min)
                ids_i32 = sb1.tile([B, 1], I32, tag="ids")
                nc.vector.tensor_copy(ids_i32[:], widf[:])

                # ---------- feedback gather + xT ----------
                if not use_mtab:
                    x_sb = sb.tile([B, E], F32, tag="x")
                    nc.gpsimd.indirect_dma_start(
                        out=x_sb[:], out_offset=None, in_=d_emb,
                        in_offset=bass.IndirectOffsetOnAxis(ap=ids_i32[:, 0:1],
                                                            axis=0))
                    xT = sb.tile([128, KC * 64], F32, tag="xT")
                    transpose_to(xT, x_sb[:])

    nc.compile()
    return nc


_BUILT = {}


def _get_nc():
    key = (T, LOGITS_F32R, USE_MTAB)
    if key not in _BUILT:
        _BUILT[key] = build(T, LOGITS_F32R, USE_MTAB)
    return _BUILT[key]


def make_in_maps(z, emb, W_proj, b_proj, W_ih, b_ih, W_hh, b_hh, W_fc, b_fc):
    z = np.asarray(z, np.float32)
    emb = np.ascontiguousarray(np.asarray(emb, np.float32))
    W_proj = np.asarray(W_proj, np.float32)
    W_ih = np.asarray(W_ih, np.float32)
    W_hh = np.asarray(W_hh, np.float32)
    W_fc = np.asarray(W_fc, np.float32)
    b_proj = np.asarray(b_proj, np.float32)
    b_ih = np.asarray(b_ih, np.float32)
    b_hh = np.asarray(b_hh, np.float32)
    b_fc = np.asarray(b_fc, np.float32)

    wihT = np.ascontiguousarray(W_ih.T)            # [512, 1536]
    whhT = np.ascontiguousarray(W_hh.T)
    wprojT = np.ascontiguousarray(W_proj.T)        # [128, 512]
    zT = np.ascontiguousarray(z.T)                 # [128, 64]
    bias_gi = b_ih.copy()
    bias_gi[0:1024] += b_hh[0:1024]
    bias_gi = bias_gi[None, :]                     # [1, 1536]
    bias_hn = b_hh[None, 1024:1536]
    bias_proj = b_proj[None, :]

    common = dict(wihT=wihT, whhT=whhT, wprojT=wprojT, zT=zT,
                  bias_gi=bias_gi, bias_hn=bias_hn, bias_proj=bias_proj)
    if USE_MTAB:
        mtab = (emb @ W_ih.T + bias_gi).astype(np.float32)
        common["mtab"] = np.ascontiguousarray(mtab)
    else:
        common["emb"] = emb

    in_maps = []
    for c in range(NC_N):
        wfc_sh = W_fc[c * VS:(c + 1) * VS, :]          # [4000, 512]
        wfcT = np.zeros((H, VSP), np.float32)
        wfcT[:, 0:VS] = wfc_sh.T
        bias_fc = np.full((1, VSP), NEG, np.float32)
        bias_fc[0, 0:VS] = b_fc[c * VS:(c + 1) * VS]
        rank_col = np.full((B, 1), float(c * VS), np.float32)
        rank_mask16 = np.zeros((B, 2 * NC_N), np.float32)
        rank_mask16[:, 2 * c:2 * c + 2] = 1.0
        m = dict(common)
        m.update(wfcT=np.ascontiguousarray(wfcT), bias_fc=bias_fc,
                 rank_col=rank_col, rank_mask16=rank_mask16)
        if LOGITS_F32R:
            m["wb"] = np.ascontiguousarray(
                np.concatenate([wfc_sh, b_fc[c * VS:(c + 1) * VS, None]],
                               axis=1))
        in_maps.append(m)
    return in_maps


def kernel(z, emb, W_proj, b_proj, W_ih, b_ih, W_hh, b_hh, W_fc, b_fc,
           context_length):
    assert int(context_length) == T
    nc = _get_nc()
    in_maps = make_in_maps(z, emb, W_proj, b_proj, W_ih, b_ih, W_hh, b_hh,
                           W_fc, b_fc)
    res = bass_utils.run_bass_kernel_spmd(nc, in_maps,
                                          core_ids=list(range(NC_N)))
    shards = [res.results[c]["out"].reshape(B, T, VS) for c in range(NC_N)]
    return np.concatenate(shards, axis=2)



# revision 35
# speedup vs baseline: 44.4202x; 5.3389x over previous
"""Trainium2 Bass kernel for nn_AutoregressiveDecoder (GRU decoder w/ greedy argmax feedback).

B=64, L=128, E=512, H=512, V=32000, T=64, 8 NeuronCores.

Sharding: vocab (V) split 8 ways; each core holds its W_fc.T shard resident in
SBUF, computes the full GRU (replicated) in fp32, its logits shard, and its
local (max, argmax).  A per-step AllGather of the 8 (max, argmax) pairs gives
every core the global argmax; feedback x = emb[ids] comes from an indirect DMA
gather out of a full emb copy in each core's DRAM.

Self-contained: hardcodes shapes; only imports the platform toolchain.
"""
import sys

if "/opt/trn_rl_repo" not in sys.path:
    sys.path.insert(0, "/opt/trn_rl_repo")

import numpy as np

import concourse.bass as bass
import concourse.mybir as mybir
import concourse.bacc as bacc
import concourse.tile as tile
import concourse.bass_utils as bass_utils
from concourse.masks import make_identity

F32 = mybir.dt.float32
F32R = mybir.dt.float32r
U32 = mybir.dt.uint32
I32 = mybir.dt.int32
AF = mybir.ActivationFunctionType
OP = mybir.AluOpType
AX = mybir.AxisListType

B, L, E, H, V, T = 64, 128, 512, 512, 32000, 64
NC_N = 8
VS = V // NC_N          # 4000 vocab per core
VSP = 4096              # padded (8 tiles of 512)
KC = H // 128           # 4 contraction chunks
NVT = VSP // 512        # 8 vocab tiles per core
NEG = -1.0e30

# ---- build flags ----
LOGITS_F32R = True      # fast fp32r logits + exact fp32 top-3 re-eval
USE_MTAB = True         # gi via gather from host-precomputed emb @ W_ih.T
SALT = 11               # dummy-input width; vary to defeat stale-NEFF caches


def _mm_acc(nc, out_ap, lhsT, rhs_list, start_first):
    """Accumulating matmul helper: sequence of (lhsT_ap, rhs_ap) into out."""
    n = len(rhs_list)
    for i, (lt, rh) in enumerate(rhs_list):
        nc.tensor.matmul(out_ap, lt, rh,
                         start=(start_first and i == 0), stop=(i == n - 1))


def build(t_steps=T, logits_f32r=LOGITS_F32R, use_mtab=USE_MTAB, no_cc=False,
          probe=None, exchange="ag"):
    # exchange: "ag" = AllGather [1,128] with PE transposes (v1-proven
    #           protocol; stable). "ar" = masked AllReduce [B,16] — faster on
    #           paper but RACES on hw (nondeterministic divergence): do not
    #           use until the collective's input-readiness is understood.
    # probe (perf-only, wrong results):
    #   "noargmax": skip local/global argmax + exchange; ids forced to 0
    #   "nogather": noargmax + mtab indirect gathers -> regular row-0 DMAs
    #   "gruonly":  nogather + skip logits matmuls (dummy output writes)
    nc = bacc.Bacc("TRN2", target_bir_lowering=False, debug=False,
                   num_devices=NC_N)

    # ---------------- DRAM I/O ----------------
    d_emb = None
    if not use_mtab:
        d_emb = nc.dram_tensor("emb", [V, E], F32, kind="ExternalInput").ap()
    d_wihT = nc.dram_tensor("wihT", [H, 3 * H], F32, kind="ExternalInput").ap()
    d_whhT = nc.dram_tensor("whhT", [H, 3 * H], F32, kind="ExternalInput").ap()
    wfc_dt = F32R if logits_f32r else F32
    d_wfcT = nc.dram_tensor("wfcT", [H, VSP], wfc_dt, kind="ExternalInput").ap()
    d_wprojT = nc.dram_tensor("wprojT", [L, H], F32, kind="ExternalInput").ap()
    d_zT = nc.dram_tensor("zT", [L, B], F32, kind="ExternalInput").ap()
    d_bias_gi = nc.dram_tensor("bias_gi", [1, 3 * H], F32, kind="ExternalInput").ap()
    d_bias_hn = nc.dram_tensor("bias_hn", [1, H], F32, kind="ExternalInput").ap()
    d_bias_fc = nc.dram_tensor("bias_fc", [1, VSP], wfc_dt, kind="ExternalInput").ap()
    d_bias_proj = nc.dram_tensor("bias_proj", [1, H], F32, kind="ExternalInput").ap()
    d_rank = nc.dram_tensor("rank_col", [B, 1], F32, kind="ExternalInput").ap()
    d_rmask = nc.dram_tensor("rank_mask16", [B, 2 * NC_N], F32,
                             kind="ExternalInput").ap()
    d_salt = nc.dram_tensor("salt", [1, max(1, int(SALT))], F32,
                            kind="ExternalInput").ap()
    if use_mtab:
        d_mtab = nc.dram_tensor("mtab", [V, 3 * H], F32, kind="ExternalInput").ap()
    if logits_f32r:
        # per-core shard of [W_fc | b_fc] for exact candidate re-evaluation
        d_wb = nc.dram_tensor("wb", [VS, E + 1], F32, kind="ExternalInput").ap()
    d_out = nc.dram_tensor("out", [B, t_steps * VS], F32, kind="ExternalOutput").ap()

    with tile.TileContext(nc) as tc:
        with tc.tile_pool(name="wts", bufs=1) as wpool, \
             tc.tile_pool(name="sb", bufs=2) as sb, \
             tc.tile_pool(name="sb1", bufs=1) as sb1, \
             tc.tile_pool(name="lgps", bufs=2, space="PSUM") as lgp, \
             tc.tile_pool(name="grups", bufs=1, space="PSUM") as grup, \
             tc.tile_pool(name="tps", bufs=2, space="PSUM") as tps, \
             tc.tile_pool(name="dr", bufs=2, space="DRAM") as dr:
            # ---------------- load weights ----------------
            wih = wpool.tile([128, KC * 3 * H], F32)          # 4x[128,1536]
            whh = wpool.tile([128, KC * 3 * H], F32)
            wfc = wpool.tile([128, KC * VSP], wfc_dt)         # 4x[128,4096]
            wproj = wpool.tile([128, H], F32)
            zT_sb = wpool.tile([128, B], F32)
            for k in range(KC):
                nc.sync.dma_start(wih[:, k * 3 * H:(k + 1) * 3 * H],
                                  d_wihT[k * 128:(k + 1) * 128, :])
                nc.sync.dma_start(whh[:, k * 3 * H:(k + 1) * 3 * H],
                                  d_whhT[k * 128:(k + 1) * 128, :])
                nc.sync.dma_start(wfc[:, k * VSP:(k + 1) * VSP],
                                  d_wfcT[k * 128:(k + 1) * 128, :])
            nc.sync.dma_start(wproj[:], d_wprojT)
            nc.sync.dma_start(zT_sb[:], d_zT)
            b_gi = wpool.tile([1, 3 * H], F32)
            b_hn = wpool.tile([1, H], F32)
            b_fc = wpool.tile([1, VSP], wfc_dt)
            b_proj = wpool.tile([1, H], F32)
            rank_col = wpool.tile([B, 1], F32)
            rmask16 = wpool.tile([B, 2 * NC_N], F32)
            nc.sync.dma_start(b_gi[:], d_bias_gi)
            nc.sync.dma_start(b_hn[:], d_bias_hn)
            nc.sync.dma_start(b_fc[:], d_bias_fc)
            nc.sync.dma_start(b_proj[:], d_bias_proj)
            nc.sync.dma_start(rank_col[:], d_rank)
            nc.sync.dma_start(rmask16[:], d_rmask)
            salt_sb = wpool.tile([1, max(1, int(SALT))], F32)
            nc.sync.dma_start(salt_sb[:], d_salt)
            ident = wpool.tile([B, B], F32)
            make_identity(nc, ident[:])
            ones1 = wpool.tile([1, 128], F32)
            nc.vector.memset(ones1[:], 1.0)

            if logits_f32r:
                ones_r = wpool.tile([1, 128], F32R)
                nc.vector.tensor_copy(ones_r[:], ones1[:])
                cand8 = wpool.tile([B, 8], F32)
                nc.vector.memset(cand8[:], NEG)
                cpair = wpool.tile([B, 8], F32)
                nc.vector.memset(cpair[:], NEG)
                cand3 = wpool.tile([B, 8], F32)
                nc.vector.memset(cand3[:], NEG)

            # ---------------- h0 ----------------
            h0_ps = lgp.tile([B, H], F32, tag="lg")
            nc.tensor.matmul(h0_ps[:], zT_sb[:], wproj[:], start=True, stop=False)
            nc.tensor.matmul(h0_ps[:], ones1[0:1, 0:B], b_proj[:],
                             start=False, stop=True)
            h_cur = sb.tile([B, H], F32, tag="h")
            nc.scalar.copy(h_cur[:], h0_ps[:])

            # transposed h (lhsT layout): [128, KC*64], chunk k at [:, 64k:64k+64]
            def transpose_to(dst_sb, src_ap, extra_dst=None):
                tp = tps.tile([128, 256], F32, tag="tp")
                for k in range(KC):
                    nc.tensor.transpose(tp[:, k * 64:(k + 1) * 64],
                                        src_ap[:, k * 128:(k + 1) * 128],
                                        ident[:])
                nc.scalar.copy(dst_sb[:], tp[:])
                if extra_dst is not None:
                    nc.vector.tensor_copy(extra_dst[:], tp[:])

            hT = sb.tile([128, KC * 64], F32, tag="hT")
            if logits_f32r:
                hT_r = sb.tile([128, KC * 64], F32R, tag="hTr")
                transpose_to(hT, h_cur[:], extra_dst=hT_r)
            else:
                hT_r = None
                transpose_to(hT, h_cur[:])

            xT = hT            # step 0: x = h0
            x_sb = None
            ids_i32 = None
            if probe is not None:
                ids_i32 = wpool.tile([B, 1], I32)
                nc.vector.memset(ids_i32[:], 0.0)
                if probe == "gruonly":
                    lg_dummy = wpool.tile([B, VS], F32)
                    nc.vector.memset(lg_dummy[:], 0.0)

            # DRAM bounce tiles for the collective
            for t in range(t_steps):
                # ---------- gh (+ rz biases) : psum ----------
                mtab_step = use_mtab and t > 0
                rz_ps = grup.tile([B, 1024], F32, tag="rz")
                ghn_ps = grup.tile([B, 512], F32, tag="ghn")
                # rz region: gh first (start); gi mms accumulate on top unless
                # this is an mtab step (gi arrives via gather + DVE add).
                for j in range(2):
                    o = rz_ps[:, j * 512:(j + 1) * 512]
                    for k in range(KC):
                        nc.tensor.matmul(o, hT[:, k * 64:(k + 1) * 64],
                                         whh[:, k * 3 * H + j * 512:
                                             k * 3 * H + (j + 1) * 512],
                                         start=(k == 0),
                                         stop=(mtab_step and k == KC - 1))
                # ghn = (h @ W_hh.T)_n + b_hh_n
                for k in range(KC):
                    nc.tensor.matmul(ghn_ps[:], hT[:, k * 64:(k + 1) * 64],
                                     whh[:, k * 3 * H + 1024:k * 3 * H + 1536],
                                     start=(k == 0), stop=False)
                nc.tensor.matmul(ghn_ps[:], ones1[0:1, 0:B], b_hn[:],
                                 start=False, stop=True)

                # ---------- gi ----------
                if mtab_step:
                    # one gather of the full mtab row [B, 1536] = gi (+b_gi),
                    # issued as soon as ids are known (overlaps gh matmuls);
                    # the rz half is added to gh_rz on DVE afterwards.
                    gall = sb1.tile([B, 3 * H], F32, tag="gall")
                    if probe in ("nogather", "gruonly"):
                        nc.sync.dma_start(gall[:], d_mtab[0:B, :])
                    else:
                        nc.gpsimd.indirect_dma_start(
                            out=gall[:], out_offset=None, in_=d_mtab,
                            in_offset=bass.IndirectOffsetOnAxis(ap=ids_i32[:, 0:1], axis=0))
                    rz_acc = sb1.tile([B, 1024], F32, tag="rzacc")
                    nc.vector.tensor_tensor(out=rz_acc[:], in0=gall[:, 0:1024],
                                            in1=rz_ps[:], op=OP.add)
                    gin_sb = gall[:, 1024:1536]
                else:
                    gin_ps = grup.tile([B, 512], F32, tag="gin")
                    for j in range(2):
                        o = rz_ps[:, j * 512:(j + 1) * 512]
                        for k in range(KC):
                            nc.tensor.matmul(o, xT[:, k * 64:(k + 1) * 64],
                                             wih[:, k * 3 * H + j * 512:
                                                 k * 3 * H + (j + 1) * 512],
                                             start=False, stop=False)
                        nc.tensor.matmul(o, ones1[0:1, 0:B],
                                         b_gi[:, j * 512:(j + 1) * 512],
                                         start=False, stop=True)
                    for k in range(KC):
                        nc.tensor.matmul(gin_ps[:], xT[:, k * 64:(k + 1) * 64],
                                         wih[:, k * 3 * H + 1024:k * 3 * H + 1536],
                                         start=(k == 0), stop=False)
                    nc.tensor.matmul(gin_ps[:], ones1[0:1, 0:B],
                                     b_gi[:, 1024:1536], start=False, stop=True)

                # ---------- gates ----------
                rz_sb = sb1.tile([B, 1024], F32, tag="rzsb")
                if mtab_step:
                    nc.scalar.activation(rz_sb[:], rz_acc[:], AF.Sigmoid)
                else:
                    nc.scalar.activation(rz_sb[:], rz_ps[:], AF.Sigmoid)
                u_sb = sb1.tile([B, H], F32, tag="u")
                nc.vector.tensor_tensor(out=u_sb[:], in0=rz_sb[:, 0:512],
                                        in1=ghn_ps[:], op=OP.mult)
                if mtab_step:
                    nc.vector.tensor_tensor(out=u_sb[:], in0=u_sb[:],
                                            in1=gall[:, 1024:1536], op=OP.add)
                else:
                    nc.vector.tensor_tensor(out=u_sb[:], in0=u_sb[:],
                                            in1=gin_ps[:], op=OP.add)
                n_sb = sb1.tile([B, H], F32, tag="n")
                nc.scalar.activation(n_sb[:], u_sb[:], AF.Tanh)
                # h_new = n + z*(h - n)
                hd_sb = sb1.tile([B, H], F32, tag="hd")
                nc.vector.tensor_tensor(out=hd_sb[:], in0=h_cur[:], in1=n_sb[:],
                                        op=OP.subtract)
                h_new = sb.tile([B, H], F32, tag="h")
                nc.vector.tensor_tensor(out=h_new[:], in0=rz_sb[:, 512:1024],
                                        in1=hd_sb[:], op=OP.mult)
                nc.vector.tensor_tensor(out=h_new[:], in0=h_new[:], in1=n_sb[:],
                                        op=OP.add)
                h_cur = h_new

                # ---------- hT ----------
                hT = sb.tile([128, KC * 64], F32, tag="hT")
                if logits_f32r:
                    hT_r = sb.tile([128, KC * 64], F32R, tag="hTr")
                    transpose_to(hT, h_cur[:], extra_dst=hT_r)
                else:
                    transpose_to(hT, h_cur[:])

                # ---------- logits ----------
                if probe == "gruonly":
                    nc.sync.dma_start(d_out[:, t * VS:(t + 1) * VS], lg_dummy[:])
                    continue
                lg_sb = sb1.tile([B, VSP], F32, tag="lg_sb")
                tm8 = sb1.tile([B, NVT * 8], F32, tag="tm8")
                lg_lhs = hT_r if logits_f32r else hT
                lg_ones = ones_r if logits_f32r else ones1
                for v in range(NVT):
                    lg_ps = lgp.tile([B, 512], F32, tag="lg")
                    for k in range(KC):
                        nc.tensor.matmul(
                            lg_ps[:], lg_lhs[:, k * 64:(k + 1) * 64],
                            wfc[:, k * VSP + v * 512:k * VSP + (v + 1) * 512],
                            start=(k == 0), stop=False)
                    nc.tensor.matmul(lg_ps[:], lg_ones[0:1, 0:B],
                                     b_fc[:, v * 512:(v + 1) * 512],
                                     start=False, stop=True)
                    nc.scalar.copy(lg_sb[:, v * 512:(v + 1) * 512], lg_ps[:])
                    nc.vector.max(out=tm8[:, v * 8:(v + 1) * 8],
                                  in_=lg_sb[:, v * 512:(v + 1) * 512])

                # DMA logits out (write only the real 4000)
                nc.sync.dma_start(d_out[:, t * VS:(t + 1) * VS], lg_sb[:, 0:VS])

                if t == t_steps - 1:
                    break       # no feedback needed after last step

                if probe is not None:
                    continue    # ids stay 0; skip argmax + exchange

                # ---------- local argmax ----------
                gmax = sb1.tile([B, 1], F32, tag="gmax")
                nc.vector.tensor_reduce(out=gmax[:], in_=tm8[:], axis=AX.X,
                                        op=OP.max)
                mi8 = sb1.tile([B, 8], U32, tag="mi8")
                if logits_f32r:
                    # fp32r logits are ~1.7e-4 loose; find top-3 candidates
                    # from the approximate logits, then re-evaluate them
                    # exactly in fp32 to pick the true argmax.
                    nc.vector.tensor_copy(cand8[:, 0:1], gmax[:])
                    v2t = sb1.tile([B, NVT * 8], F32, tag="v2t")
                    nc.vector.match_replace(out=v2t[:], in_to_replace=cand8[:, 0:8],
                                            in_values=tm8[:], imm_value=NEG)
                    v2v = sb1.tile([B, 1], F32, tag="v2v")
                    nc.vector.tensor_reduce(out=v2v[:], in_=v2t[:], axis=AX.X,
                                            op=OP.max)
                    nc.vector.tensor_copy(cpair[:, 0:1], v2v[:])
                    v3t = sb1.tile([B, NVT * 8], F32, tag="v3t")
                    nc.vector.match_replace(out=v3t[:], in_to_replace=cpair[:, 0:8],
                                            in_values=v2t[:], imm_value=NEG)
                    v3v = sb1.tile([B, 1], F32, tag="v3v")
                    nc.vector.tensor_reduce(out=v3v[:], in_=v3t[:], axis=AX.X,
                                            op=OP.max)
                    nc.vector.tensor_copy(cand3[:, 0:1], gmax[:])
                    nc.vector.tensor_copy(cand3[:, 1:2], v2v[:])
                    nc.vector.tensor_copy(cand3[:, 2:3], v3v[:])
                    nc.vector.max_index(out=mi8[:], in_max=cand3[:],
                                        in_values=lg_sb[:])
                    idl = sb1.tile([B, 8], I32, tag="idl")
                    nc.vector.tensor_copy(idl[:, 0:3], mi8[:, 0:3])
                    wb3 = sb1.tile([B, 3 * (E + 1)], F32, tag="wb3")
                    p3 = sb1.tile([B, 3 * E], F32, tag="p3")
                    # 3 single-offset gathers: a single multi-offset gather
                    # (offset ap [B,3]) passes CoreSim but mis-fills on hw.
                    for j in range(3):
                        nc.gpsimd.indirect_dma_start(
                            out=wb3[:, j * (E + 1):(j + 1) * (E + 1)],
                            out_offset=None, in_=d_wb,
                            in_offset=bass.IndirectOffsetOnAxis(
                                ap=idl[:, j:j + 1], axis=0))
                        nc.vector.tensor_tensor(
                            out=p3[:, j * E:(j + 1) * E], in0=h_cur[:],
                            in1=wb3[:, j * (E + 1):j * (E + 1) + E], op=OP.mult)
                    e3 = sb1.tile([B, 4], F32, tag="e3")
                    nc.vector.tensor_reduce(
                        out=e3[:, 0:3],
                        in_=p3[:].rearrange("p (j e) -> p j e", j=3),
                        axis=AX.X, op=OP.add)
                    # add per-candidate bias (wb3 col E of each 513-block)
                    nc.vector.tensor_tensor(
                        out=e3[:, 0:3], in0=e3[:, 0:3],
                        in1=wb3[:].rearrange("p (j e) -> p j e", j=3)[:, :, E:E + 1].squeeze(),
                        op=OP.add)
                    idf = sb1.tile([B, 8], F32, tag="idf")
                    nc.vector.tensor_copy(idf[:, 0:3], mi8[:, 0:3])
                    nc.vector.tensor_scalar(out=idf[:, 0:3], in0=idf[:, 0:3],
                                            scalar1=rank_col[:, 0:1],
                                            scalar2=None, op0=OP.add)
                    cmp01 = sb1.tile([B, 1], I32, tag="cmp01")
                    nc.vector.tensor_tensor(out=cmp01[:], in0=e3[:, 1:2],
                                            in1=e3[:, 0:1], op=OP.is_gt)
                    m01 = sb1.tile([B, 1], F32, tag="m01")
                    nc.vector.tensor_tensor(out=m01[:], in0=e3[:, 0:1],
                                            in1=e3[:, 1:2], op=OP.max)
                    g01 = sb1.tile([B, 1], F32, tag="g01")
                    nc.vector.select(out=g01[:], mask=cmp01[:],
                                     on_true=idf[:, 1:2], on_false=idf[:, 0:1])
                    cmp2 = sb1.tile([B, 1], I32, tag="cmp2")
                    nc.vector.tensor_tensor(out=cmp2[:], in0=e3[:, 2:3],
                                            in1=m01[:], op=OP.is_gt)
                    pay2 = sb1.tile([B, 2], F32, tag="pay2")
                    nc.vector.tensor_tensor(out=pay2[:, 0:1], in0=m01[:],
                                            in1=e3[:, 2:3], op=OP.max)
                    nc.vector.select(out=pay2[:, 1:2], mask=cmp2[:],
                                     on_true=idf[:, 2:3], on_false=g01[:])
                else:
                    gmax8 = sb1.tile([B, 8], F32, tag="gmax8")
                    nc.vector.tensor_copy(gmax8[:], gmax[:].to_broadcast([B, 8]))
                    nc.vector.max_index(out=mi8[:], in_max=gmax8[:],
                                        in_values=lg_sb[:])
                    pay2 = sb1.tile([B, 2], F32, tag="pay2")
                    nc.vector.tensor_copy(pay2[:, 0:1], gmax[:])
                    nc.vector.tensor_copy(pay2[:, 1:2], mi8[:, 0:1])
                    nc.vector.tensor_scalar(out=pay2[:, 1:2], in0=pay2[:, 1:2],
                                            scalar1=rank_col[:, 0:1], scalar2=None,
                                            op0=OP.add)

                # ---------- cross-core exchange of (emax, gid) pairs ----------
                if exchange == "ar":
                    # masked AllReduce(add): core r contributes its pair only
                    # in cols (2r, 2r+1); the sum assembles the full 8-core
                    # table on every core, no transposes.
                    cc16 = sb1.tile([B, 2 * NC_N], F32, tag="cc16")
                    nc.vector.tensor_tensor(
                        out=cc16[:].rearrange("p (r t) -> p r t", r=NC_N),
                        in0=pay2[:].rearrange("p (o t) -> p o t", o=1)
                            .to_broadcast([B, NC_N, 2]),
                        in1=rmask16[:].rearrange("p (r t) -> p r t", r=NC_N),
                        op=OP.mult)
                    cc_in = dr.tile([B, 2 * NC_N], F32, tag="ccin")
                    cc_out = dr.tile([B, 2 * NC_N], F32, tag="ccout")
                    nc.sync.dma_start(cc_in[:], cc16[:])
                    if no_cc:
                        # perf probe only (wrong results): skip the collective
                        nc.sync.dma_start(cc_out[:], cc_in[:])
                    else:
                        nc.gpsimd.collective_compute(
                            "AllReduce", OP.add,
                            replica_groups=[list(range(NC_N))],
                            ins=[cc_in[:].opt()], outs=[cc_out[:].opt()])
                    ag16 = sb1.tile([B, 2 * NC_N], F32, tag="agsb")
                    nc.sync.dma_start(ag16[:], cc_out[:])
                    vag = ag16[:].rearrange("p (r t) -> p r t", r=NC_N)
                else:
                    # v1-style AllGather of a [1,128] row (pair-transposed)
                    payT_ps = tps.tile([128, 256], F32, tag="tp")
                    nc.tensor.transpose(payT_ps[0:1, 0:64], pay2[:, 0:1],
                                        ident[:])
                    nc.tensor.transpose(payT_ps[0:1, 64:128], pay2[:, 1:2],
                                        ident[:])
                    pay_row = sb1.tile([1, 128], F32, tag="payrow")
                    nc.vector.tensor_copy(pay_row[:], payT_ps[0:1, 0:128])
                    cc_in = dr.tile([1, 128], F32, tag="ccin")
                    cc_out = dr.tile([NC_N, 128], F32, tag="ccout")
                    # bounce DMAs MUST stay on the gpsimd queue: same-queue
                    # ordering with collective_compute is what serializes
                    # write->collective->read on hw.
                    nc.gpsimd.dma_start(cc_in[:], pay_row[:])
                    if no_cc:
                        for rr in range(NC_N):
                            nc.gpsimd.dma_start(cc_out[rr:rr + 1, :], cc_in[:])
                    else:
                        nc.gpsimd.collective_compute(
                            "AllGather", OP.bypass,
                            replica_groups=[list(range(NC_N))],
                            ins=[cc_in[:].opt()], outs=[cc_out[:].opt()])
                    ag_sb = sb1.tile([NC_N, 128], F32, tag="agsb")
                    nc.gpsimd.dma_start(ag_sb[:], cc_out[:])
                    agT_ps = tps.tile([128, 256], F32, tag="tp")
                    nc.tensor.transpose(agT_ps[0:B, 0:8], ag_sb[:, 0:64],
                                        ident[0:8, 0:8])
                    nc.tensor.transpose(agT_ps[0:B, 8:16], ag_sb[:, 64:128],
                                        ident[0:8, 0:8])
                    agT = sb1.tile([B, 16], F32, tag="agTsb")
                    nc.vector.tensor_copy(agT[:], agT_ps[0:B, 0:16])
                    # view cols (r, j): maxes at [:, 0:8], ids at [:, 8:16]
                    vag = agT[:].rearrange("p (t r) -> p r t", t=2)

                # ---------- global argmax ----------
                gm = sb1.tile([B, 1], F32, tag="gm")
                nc.vector.tensor_reduce(out=gm[:], in_=vag[:, :, 0:1].squeeze(),
                                        axis=AX.X, op=OP.max)
                mask = sb1.tile([B, 8], F32, tag="mask")
                nc.vector.tensor_scalar(out=mask[:], in0=vag[:, :, 0:1].squeeze(),
                                        scalar1=gm[:, 0:1], scalar2=None,
                                        op0=OP.is_lt)    # 1.0 where NOT max
                sel = sb1.tile([B, 8], F32, tag="sel")
                # sel = gid + (not-max)*BIG  -> min over ranks = winning gid
                nc.vector.tensor_scalar(out=sel[:], in0=mask[:],
                                        scalar1=2.0e9, scalar2=None,
                                        op0=OP.mult)
                nc.vector.tensor_tensor(out=sel[:], in0=sel[:],
                                        in1=vag[:, :, 1:2].squeeze(), op=OP.add)
                widf = sb1.tile([B, 1], F32, tag="widf")
                nc.vector.tensor_reduce(out=widf[:], in_=sel[:], axis=AX.X,
                                        op=OP.min)
                ids_i32 = sb1.tile([B, 1], I32, tag="ids")
                nc.vector.tensor_copy(ids_i32[:], widf[:])

                # ---------- feedback gather + xT ----------
                if not use_mtab:
                    x_sb = sb.tile([B, E], F32, tag="x")
                    nc.gpsimd.indirect_dma_start(
                        out=x_sb[:], out_offset=None, in_=d_emb,
                        in_offset=bass.IndirectOffsetOnAxis(ap=ids_i32[:, 0:1],
                                                            axis=0))
                    xT = sb.tile([128, KC * 64], F32, tag="xT")
                    transpose_to(xT, x_sb[:])

    nc.compile()
    return nc


_BUILT = {}


def _get_nc():
    key = (T, LOGITS_F32R, USE_MTAB)
    if key not in _BUILT:
        _BUILT[key] = build(T, LOGITS_F32R, USE_MTAB)
    return _BUILT[key]


def make_in_maps(z, emb, W_proj, b_proj, W_ih, b_ih, W_hh, b_hh, W_fc, b_fc):
    z = np.asarray(z, np.float32)
    emb = np.ascontiguousarray(np.asarray(emb, np.float32))
    W_proj = np.asarray(W_proj, np.float32)
    W_ih = np.asarray(W_ih, np.float32)
    W_hh = np.asarray(W_hh, np.float32)
    W_fc = np.asarray(W_fc, np.float32)
    b_proj = np.asarray(b_proj, np.float32)
    b_ih = np.asarray(b_ih, np.float32)
    b_hh = np.asarray(b_hh, np.float32)
    b_fc = np.asarray(b_fc, np.float32)

    wihT = np.ascontiguousarray(W_ih.T)            # [512, 1536]
    whhT = np.ascontiguousarray(W_hh.T)
    wprojT = np.ascontiguousarray(W_proj.T)        # [128, 512]
    zT = np.ascontiguousarray(z.T)                 # [128, 64]
    bias_gi = b_ih.copy()
    bias_gi[0:1024] += b_hh[0:1024]
    bias_gi = bias_gi[None, :]                     # [1, 1536]
    bias_hn = b_hh[None, 1024:1536]
    bias_proj = b_proj[None, :]

    common = dict(wihT=wihT, whhT=whhT, wprojT=wprojT, zT=zT,
                  bias_gi=bias_gi, bias_hn=bias_hn, bias_proj=bias_proj,
                  salt=np.zeros((1, max(1, int(SALT))), np.float32))
    if USE_MTAB:
        mtab = (emb @ W_ih.T + bias_gi).astype(np.float32)
        common["mtab"] = np.ascontiguousarray(mtab)
    else:
        common["emb"] = emb

    in_maps = []
    for c in range(NC_N):
        wfc_sh = W_fc[c * VS:(c + 1) * VS, :]          # [4000, 512]
        wfcT = np.zeros((H, VSP), np.float32)
        wfcT[:, 0:VS] = wfc_sh.T
        bias_fc = np.full((1, VSP), NEG, np.float32)
        bias_fc[0, 0:VS] = b_fc[c * VS:(c + 1) * VS]
        rank_col = np.full((B, 1), float(c * VS), np.float32)
        rank_mask16 = np.zeros((B, 2 * NC_N), np.float32)
        rank_mask16[:, 2 * c:2 * c + 2] = 1.0
        m = dict(common)
        m.update(wfcT=np.ascontiguousarray(wfcT), bias_fc=bias_fc,
                 rank_col=rank_col, rank_mask16=rank_mask16)
        if LOGITS_F32R:
            m["wb"] = np.ascontiguousarray(
                np.concatenate([wfc_sh, b_fc[c * VS:(c + 1) * VS, None]],
                               axis=1))
        in_maps.append(m)
    return in_maps


def kernel(z, emb, W_proj, b_proj, W_ih, b_ih, W_hh, b_hh, W_fc, b_fc,
           context_length):
    assert int(context_length) == T
    nc = _get_nc()
    in_maps = make_in_maps(z, emb, W_proj, b_proj, W_ih, b_ih, W_hh, b_hh,
                           W_fc, b_fc)
    res = bass_utils.run_bass_kernel_spmd(nc, in_maps,
                                          core_ids=list(range(NC_N)))
    shards = [res.results[c]["out"].reshape(B, T, VS) for c in range(NC_N)]
    return np.concatenate(shards, axis=2)



# revision 39
# speedup vs baseline: 46.9010x; 1.0558x over previous
"""Trainium2 Bass kernel for nn_AutoregressiveDecoder (GRU decoder w/ greedy argmax feedback).

B=64, L=128, E=512, H=512, V=32000, T=64, 8 NeuronCores.

Sharding: vocab (V) split 8 ways; each core holds its W_fc.T shard resident in
SBUF, computes the full GRU (replicated) in fp32, its logits shard, and its
local (max, argmax).  A per-step AllGather of the 8 (max, argmax) pairs gives
every core the global argmax; feedback x = emb[ids] comes from an indirect DMA
gather out of a full emb copy in each core's DRAM.

Self-contained: hardcodes shapes; only imports the platform toolchain.
"""
import sys

if "/opt/trn_rl_repo" not in sys.path:
    sys.path.insert(0, "/opt/trn_rl_repo")

import numpy as np

import concourse.bass as bass
import concourse.mybir as mybir
import concourse.bacc as bacc
import concourse.tile as tile
import concourse.bass_utils as bass_utils
from concourse.masks import make_identity

F32 = mybir.dt.float32
F32R = mybir.dt.float32r
U32 = mybir.dt.uint32
I32 = mybir.dt.int32
AF = mybir.ActivationFunctionType
OP = mybir.AluOpType
AX = mybir.AxisListType

B, L, E, H, V, T = 64, 128, 512, 512, 32000, 64
NC_N = 8
VS = V // NC_N          # 4000 vocab per core
VSP = 4096              # padded (8 tiles of 512)
KC = H // 128           # 4 contraction chunks
NVT = VSP // 512        # 8 vocab tiles per core
NEG = -1.0e30

# ---- build flags ----
LOGITS_F32R = True      # fast fp32r logits + exact fp32 top-3 re-eval
USE_MTAB = True         # gi via gather from host-precomputed emb @ W_ih.T
SALT = 13               # dummy-input width; vary to defeat stale-NEFF caches


def _mm_acc(nc, out_ap, lhsT, rhs_list, start_first):
    """Accumulating matmul helper: sequence of (lhsT_ap, rhs_ap) into out."""
    n = len(rhs_list)
    for i, (lt, rh) in enumerate(rhs_list):
        nc.tensor.matmul(out_ap, lt, rh,
                         start=(start_first and i == 0), stop=(i == n - 1))


def build(t_steps=T, logits_f32r=LOGITS_F32R, use_mtab=USE_MTAB, no_cc=False,
          probe=None, exchange="ag"):
    # exchange: "ag" = AllGather [1,128] with PE transposes (v1-proven
    #           protocol; stable). "ar" = masked AllReduce [B,16] — faster on
    #           paper but RACES on hw (nondeterministic divergence): do not
    #           use until the collective's input-readiness is understood.
    # probe (perf-only, wrong results):
    #   "noargmax": skip local/global argmax + exchange; ids forced to 0
    #   "nogather": noargmax + mtab indirect gathers -> regular row-0 DMAs
    #   "gruonly":  nogather + skip logits matmuls (dummy output writes)
    nc = bacc.Bacc("TRN2", target_bir_lowering=False, debug=False,
                   num_devices=NC_N)

    # ---------------- DRAM I/O ----------------
    d_emb = None
    if not use_mtab:
        d_emb = nc.dram_tensor("emb", [V, E], F32, kind="ExternalInput").ap()
    d_wihT = nc.dram_tensor("wihT", [H, 3 * H], F32, kind="ExternalInput").ap()
    d_whhT = nc.dram_tensor("whhT", [H, 3 * H], F32, kind="ExternalInput").ap()
    wfc_dt = F32R if logits_f32r else F32
    d_wfcT = nc.dram_tensor("wfcT", [H, VSP], wfc_dt, kind="ExternalInput").ap()
    d_wprojT = nc.dram_tensor("wprojT", [L, H], F32, kind="ExternalInput").ap()
    d_zT = nc.dram_tensor("zT", [L, B], F32, kind="ExternalInput").ap()
    d_bias_gi = nc.dram_tensor("bias_gi", [1, 3 * H], F32, kind="ExternalInput").ap()
    d_bias_hn = nc.dram_tensor("bias_hn", [1, H], F32, kind="ExternalInput").ap()
    d_bias_fc = nc.dram_tensor("bias_fc", [1, VSP], wfc_dt, kind="ExternalInput").ap()
    d_bias_proj = nc.dram_tensor("bias_proj", [1, H], F32, kind="ExternalInput").ap()
    d_rank = nc.dram_tensor("rank_col", [B, 1], F32, kind="ExternalInput").ap()
    d_rmask = nc.dram_tensor("rank_mask16", [B, 2 * NC_N], F32,
                             kind="ExternalInput").ap()
    d_salt = nc.dram_tensor("salt", [1, max(1, int(SALT))], F32,
                            kind="ExternalInput").ap()
    if use_mtab:
        d_mtab = nc.dram_tensor("mtab", [V, 3 * H], F32, kind="ExternalInput").ap()
    if logits_f32r:
        # per-core shard of [W_fc | b_fc] for exact candidate re-evaluation
        d_wb = nc.dram_tensor("wb", [VS, E + 1], F32, kind="ExternalInput").ap()
    d_out = nc.dram_tensor("out", [B, t_steps * VS], F32, kind="ExternalOutput").ap()

    with tile.TileContext(nc) as tc:
        with tc.tile_pool(name="wts", bufs=1) as wpool, \
             tc.tile_pool(name="sb", bufs=2) as sb, \
             tc.tile_pool(name="sb1", bufs=1) as sb1, \
             tc.tile_pool(name="lgps", bufs=2, space="PSUM") as lgp, \
             tc.tile_pool(name="grups", bufs=1, space="PSUM") as grup, \
             tc.tile_pool(name="tps", bufs=2, space="PSUM") as tps, \
             tc.tile_pool(name="dr", bufs=2, space="DRAM") as dr:
            # ---------------- load weights ----------------
            wih = wpool.tile([128, KC * 3 * H], F32)          # 4x[128,1536]
            whh = wpool.tile([128, KC * 3 * H], F32)
            wfc = wpool.tile([128, KC * VSP], wfc_dt)         # 4x[128,4096]
            wproj = wpool.tile([128, H], F32)
            zT_sb = wpool.tile([128, B], F32)
            for k in range(KC):
                nc.sync.dma_start(wih[:, k * 3 * H:(k + 1) * 3 * H],
                                  d_wihT[k * 128:(k + 1) * 128, :])
                nc.sync.dma_start(whh[:, k * 3 * H:(k + 1) * 3 * H],
                                  d_whhT[k * 128:(k + 1) * 128, :])
                nc.sync.dma_start(wfc[:, k * VSP:(k + 1) * VSP],
                                  d_wfcT[k * 128:(k + 1) * 128, :])
            nc.sync.dma_start(wproj[:], d_wprojT)
            nc.sync.dma_start(zT_sb[:], d_zT)
            b_gi = wpool.tile([1, 3 * H], F32)
            b_hn = wpool.tile([1, H], F32)
            b_fc = wpool.tile([1, VSP], wfc_dt)
            b_proj = wpool.tile([1, H], F32)
            rank_col = wpool.tile([B, 1], F32)
            rmask16 = wpool.tile([B, 2 * NC_N], F32)
            nc.sync.dma_start(b_gi[:], d_bias_gi)
            nc.sync.dma_start(b_hn[:], d_bias_hn)
            nc.sync.dma_start(b_fc[:], d_bias_fc)
            nc.sync.dma_start(b_proj[:], d_bias_proj)
            nc.sync.dma_start(rank_col[:], d_rank)
            nc.sync.dma_start(rmask16[:], d_rmask)
            salt_sb = wpool.tile([1, max(1, int(SALT))], F32)
            nc.sync.dma_start(salt_sb[:], d_salt)
            ident = wpool.tile([B, B], F32)
            make_identity(nc, ident[:])
            ones1 = wpool.tile([1, 128], F32)
            nc.vector.memset(ones1[:], 1.0)

            if logits_f32r:
                ones_r = wpool.tile([1, 128], F32R)
                nc.vector.tensor_copy(ones_r[:], ones1[:])
                cand8 = wpool.tile([B, 8], F32)
                nc.vector.memset(cand8[:], NEG)
                cpair = wpool.tile([B, 8], F32)
                nc.vector.memset(cpair[:], NEG)
                cand3 = wpool.tile([B, 8], F32)
                nc.vector.memset(cand3[:], NEG)

            # ---------------- h0 ----------------
            h0_ps = lgp.tile([B, H], F32, tag="lg")
            nc.tensor.matmul(h0_ps[:], zT_sb[:], wproj[:], start=True, stop=False)
            nc.tensor.matmul(h0_ps[:], ones1[0:1, 0:B], b_proj[:],
                             start=False, stop=True)
            h_cur = sb.tile([B, H], F32, tag="h")
            nc.scalar.copy(h_cur[:], h0_ps[:])

            # transposed h (lhsT layout): [128, KC*64], chunk k at [:, 64k:64k+64]
            def transpose_to(dst_sb, src_ap, extra_dst=None):
                tp = tps.tile([128, 256], F32, tag="tp")
                for k in range(KC):
                    nc.tensor.transpose(tp[:, k * 64:(k + 1) * 64],
                                        src_ap[:, k * 128:(k + 1) * 128],
                                        ident[:])
                nc.scalar.copy(dst_sb[:], tp[:])
                if extra_dst is not None:
                    nc.vector.tensor_copy(extra_dst[:], tp[:])

            hT = sb.tile([128, KC * 64], F32, tag="hT")
            if logits_f32r:
                hT_r = sb.tile([128, KC * 64], F32R, tag="hTr")
                transpose_to(hT, h_cur[:], extra_dst=hT_r)
            else:
                hT_r = None
                transpose_to(hT, h_cur[:])

            xT = hT            # step 0: x = h0
            x_sb = None
            ids_i32 = None
            if probe is not None:
                ids_i32 = wpool.tile([B, 1], I32)
                nc.vector.memset(ids_i32[:], 0.0)
                if probe == "gruonly":
                    lg_dummy = wpool.tile([B, VS], F32)
                    nc.vector.memset(lg_dummy[:], 0.0)

            # DRAM bounce tiles for the collective
            for t in range(t_steps):
                # ---------- gh (+ rz biases) : psum ----------
                mtab_step = use_mtab and t > 0
                rz_ps = grup.tile([B, 1024], F32, tag="rz")
                ghn_ps = grup.tile([B, 512], F32, tag="ghn")
                # rz region: gh first (start); gi mms accumulate on top unless
                # this is an mtab step (gi arrives via gather + DVE add).
                for j in range(2):
                    o = rz_ps[:, j * 512:(j + 1) * 512]
                    for k in range(KC):
                        nc.tensor.matmul(o, hT[:, k * 64:(k + 1) * 64],
                                         whh[:, k * 3 * H + j * 512:
                                             k * 3 * H + (j + 1) * 512],
                                         start=(k == 0),
                                         stop=(mtab_step and k == KC - 1))
                # ghn = (h @ W_hh.T)_n + b_hh_n
                for k in range(KC):
                    nc.tensor.matmul(ghn_ps[:], hT[:, k * 64:(k + 1) * 64],
                                     whh[:, k * 3 * H + 1024:k * 3 * H + 1536],
                                     start=(k == 0), stop=False)
                nc.tensor.matmul(ghn_ps[:], ones1[0:1, 0:B], b_hn[:],
                                 start=False, stop=True)

                # ---------- gi ----------
                if mtab_step:
                    # one gather of the full mtab row [B, 1536] = gi (+b_gi),
                    # issued as soon as ids are known (overlaps gh matmuls);
                    # the rz half is added to gh_rz on DVE afterwards.
                    gall = sb1.tile([B, 3 * H], F32, tag="gall")
                    if probe in ("nogather", "gruonly"):
                        nc.sync.dma_start(gall[:], d_mtab[0:B, :])
                    else:
                        nc.gpsimd.indirect_dma_start(
                            out=gall[:], out_offset=None, in_=d_mtab,
                            in_offset=bass.IndirectOffsetOnAxis(ap=ids_i32[:, 0:1], axis=0))
                    rz_acc = sb1.tile([B, 1024], F32, tag="rzacc")
                    nc.vector.tensor_tensor(out=rz_acc[:], in0=gall[:, 0:1024],
                                            in1=rz_ps[:], op=OP.add)
                    gin_sb = gall[:, 1024:1536]
                else:
                    gin_ps = grup.tile([B, 512], F32, tag="gin")
                    for j in range(2):
                        o = rz_ps[:, j * 512:(j + 1) * 512]
                        for k in range(KC):
                            nc.tensor.matmul(o, xT[:, k * 64:(k + 1) * 64],
                                             wih[:, k * 3 * H + j * 512:
                                                 k * 3 * H + (j + 1) * 512],
                                             start=False, stop=False)
                        nc.tensor.matmul(o, ones1[0:1, 0:B],
                                         b_gi[:, j * 512:(j + 1) * 512],
                                         start=False, stop=True)
                    for k in range(KC):
                        nc.tensor.matmul(gin_ps[:], xT[:, k * 64:(k + 1) * 64],
                                         wih[:, k * 3 * H + 1024:k * 3 * H + 1536],
                                         start=(k == 0), stop=False)
                    nc.tensor.matmul(gin_ps[:], ones1[0:1, 0:B],
                                     b_gi[:, 1024:1536], start=False, stop=True)

                # ---------- gates ----------
                # r-sigmoid first: it alone gates the u->tanh critical path;
                # z-sigmoid runs in tanh's shadow.
                rz_sb = sb1.tile([B, 1024], F32, tag="rzsb")
                rz_src = rz_acc if mtab_step else rz_ps
                nc.scalar.activation(rz_sb[:, 0:512], rz_src[:, 0:512],
                                     AF.Sigmoid)
                u_sb = sb1.tile([B, H], F32, tag="u")
                nc.vector.tensor_tensor(out=u_sb[:], in0=rz_sb[:, 0:512],
                                        in1=ghn_ps[:], op=OP.mult)
                nc.scalar.activation(rz_sb[:, 512:1024], rz_src[:, 512:1024],
                                     AF.Sigmoid)
                if mtab_step:
                    nc.vector.tensor_tensor(out=u_sb[:], in0=u_sb[:],
                                            in1=gall[:, 1024:1536], op=OP.add)
                else:
                    nc.vector.tensor_tensor(out=u_sb[:], in0=u_sb[:],
                                            in1=gin_ps[:], op=OP.add)
                n_sb = sb1.tile([B, H], F32, tag="n")
                nc.scalar.activation(n_sb[:], u_sb[:], AF.Tanh)
                # h_new = n + z*(h - n)
                hd_sb = sb1.tile([B, H], F32, tag="hd")
                nc.vector.tensor_tensor(out=hd_sb[:], in0=h_cur[:], in1=n_sb[:],
                                        op=OP.subtract)
                h_new = sb.tile([B, H], F32, tag="h")
                nc.vector.tensor_tensor(out=h_new[:], in0=rz_sb[:, 512:1024],
                                        in1=hd_sb[:], op=OP.mult)
                nc.vector.tensor_tensor(out=h_new[:], in0=h_new[:], in1=n_sb[:],
                                        op=OP.add)
                h_cur = h_new

                # ---------- hT ----------
                hT = sb.tile([128, KC * 64], F32, tag="hT")
                if logits_f32r:
                    hT_r = sb.tile([128, KC * 64], F32R, tag="hTr")
                    transpose_to(hT, h_cur[:], extra_dst=hT_r)
                else:
                    transpose_to(hT, h_cur[:])

                # ---------- logits ----------
                if probe == "gruonly":
                    nc.sync.dma_start(d_out[:, t * VS:(t + 1) * VS], lg_dummy[:])
                    continue
                lg_sb = sb1.tile([B, VSP], F32, tag="lg_sb")
                tm8 = sb1.tile([B, NVT * 8], F32, tag="tm8")
                lg_lhs = hT_r if logits_f32r else hT
                lg_ones = ones_r if logits_f32r else ones1
                for v in range(NVT):
                    lg_ps = lgp.tile([B, 512], F32, tag="lg")
                    for k in range(KC):
                        nc.tensor.matmul(
                            lg_ps[:], lg_lhs[:, k * 64:(k + 1) * 64],
                            wfc[:, k * VSP + v * 512:k * VSP + (v + 1) * 512],
                            start=(k == 0), stop=False)
                    nc.tensor.matmul(lg_ps[:], lg_ones[0:1, 0:B],
                                     b_fc[:, v * 512:(v + 1) * 512],
                                     start=False, stop=True)
                    nc.scalar.copy(lg_sb[:, v * 512:(v + 1) * 512], lg_ps[:])
                    nc.vector.max(out=tm8[:, v * 8:(v + 1) * 8],
                                  in_=lg_sb[:, v * 512:(v + 1) * 512])

                # DMA logits out (write only the real 4000)
                nc.sync.dma_start(d_out[:, t * VS:(t + 1) * VS], lg_sb[:, 0:VS])

                if t == t_steps - 1:
                    break       # no feedback needed after last step

                if probe is not None:
                    continue    # ids stay 0; skip argmax + exchange

                # ---------- local argmax ----------
                mi8 = sb1.tile([B, 8], U32, tag="mi8")
                if logits_f32r:
                    # fp32r logits are ~1.7e-4 loose; find top-3 candidates
                    # from the approximate logits, then re-evaluate them
                    # exactly in fp32 to pick the true argmax. The three
                    # reduces write cand3 cols 0..2 directly.
                    nc.vector.tensor_reduce(out=cand3[:, 0:1], in_=tm8[:],
                                            axis=AX.X, op=OP.max)
                    nc.vector.tensor_copy(cand8[:, 0:1], cand3[:, 0:1])
                    v2t = sb1.tile([B, NVT * 8], F32, tag="v2t")
                    nc.vector.match_replace(out=v2t[:], in_to_replace=cand8[:, 0:8],
                                            in_values=tm8[:], imm_value=NEG)
                    nc.vector.tensor_reduce(out=cand3[:, 1:2], in_=v2t[:],
                                            axis=AX.X, op=OP.max)
                    nc.vector.tensor_copy(cpair[:, 0:1], cand3[:, 1:2])
                    v3t = sb1.tile([B, NVT * 8], F32, tag="v3t")
                    nc.vector.match_replace(out=v3t[:], in_to_replace=cpair[:, 0:8],
                                            in_values=v2t[:], imm_value=NEG)
                    nc.vector.tensor_reduce(out=cand3[:, 2:3], in_=v3t[:],
                                            axis=AX.X, op=OP.max)
                    nc.vector.max_index(out=mi8[:], in_max=cand3[:],
                                        in_values=lg_sb[:])
                    idl = sb1.tile([B, 8], I32, tag="idl")
                    nc.vector.tensor_copy(idl[:, 0:3], mi8[:, 0:3])
                    wb3 = sb1.tile([B, 3 * (E + 1)], F32, tag="wb3")
                    p3 = sb1.tile([B, 3 * E], F32, tag="p3")
                    # 3 single-offset gathers: a single multi-offset gather
                    # (offset ap [B,3]) passes CoreSim but mis-fills on hw.
                    for j in range(3):
                        nc.gpsimd.indirect_dma_start(
                            out=wb3[:, j * (E + 1):(j + 1) * (E + 1)],
                            out_offset=None, in_=d_wb,
                            in_offset=bass.IndirectOffsetOnAxis(
                                ap=idl[:, j:j + 1], axis=0))
                        nc.vector.tensor_tensor(
                            out=p3[:, j * E:(j + 1) * E], in0=h_cur[:],
                            in1=wb3[:, j * (E + 1):j * (E + 1) + E], op=OP.mult)
                    e3 = sb1.tile([B, 4], F32, tag="e3")
                    nc.vector.tensor_reduce(
                        out=e3[:, 0:3],
                        in_=p3[:].rearrange("p (j e) -> p j e", j=3),
                        axis=AX.X, op=OP.add)
                    # add per-candidate bias (wb3 col E of each 513-block)
                    nc.vector.tensor_tensor(
                        out=e3[:, 0:3], in0=e3[:, 0:3],
                        in1=wb3[:].rearrange("p (j e) -> p j e", j=3)[:, :, E:E + 1].squeeze(),
                        op=OP.add)
                    idf = sb1.tile([B, 8], F32, tag="idf")
                    nc.vector.tensor_copy(idf[:, 0:3], mi8[:, 0:3])
                    nc.vector.tensor_scalar(out=idf[:, 0:3], in0=idf[:, 0:3],
                                            scalar1=rank_col[:, 0:1],
                                            scalar2=None, op0=OP.add)
                    cmp01 = sb1.tile([B, 1], I32, tag="cmp01")
                    nc.vector.tensor_tensor(out=cmp01[:], in0=e3[:, 1:2],
                                            in1=e3[:, 0:1], op=OP.is_gt)
                    m01 = sb1.tile([B, 1], F32, tag="m01")
                    nc.vector.tensor_tensor(out=m01[:], in0=e3[:, 0:1],
                                            in1=e3[:, 1:2], op=OP.max)
                    g01 = sb1.tile([B, 1], F32, tag="g01")
                    nc.vector.select(out=g01[:], mask=cmp01[:],
                                     on_true=idf[:, 1:2], on_false=idf[:, 0:1])
                    cmp2 = sb1.tile([B, 1], I32, tag="cmp2")
                    nc.vector.tensor_tensor(out=cmp2[:], in0=e3[:, 2:3],
                                            in1=m01[:], op=OP.is_gt)
                    pay2 = sb1.tile([B, 2], F32, tag="pay2")
                    nc.vector.tensor_tensor(out=pay2[:, 0:1], in0=m01[:],
                                            in1=e3[:, 2:3], op=OP.max)
                    nc.vector.select(out=pay2[:, 1:2], mask=cmp2[:],
                                     on_true=idf[:, 2:3], on_false=g01[:])
                else:
                    gmax = sb1.tile([B, 1], F32, tag="gmax")
                    nc.vector.tensor_reduce(out=gmax[:], in_=tm8[:], axis=AX.X,
                                            op=OP.max)
                    gmax8 = sb1.tile([B, 8], F32, tag="gmax8")
                    nc.vector.tensor_copy(gmax8[:], gmax[:].to_broadcast([B, 8]))
                    nc.vector.max_index(out=mi8[:], in_max=gmax8[:],
                                        in_values=lg_sb[:])
                    pay2 = sb1.tile([B, 2], F32, tag="pay2")
                    nc.vector.tensor_copy(pay2[:, 0:1], gmax[:])
                    nc.vector.tensor_copy(pay2[:, 1:2], mi8[:, 0:1])
                    nc.vector.tensor_scalar(out=pay2[:, 1:2], in0=pay2[:, 1:2],
                                            scalar1=rank_col[:, 0:1], scalar2=None,
                                            op0=OP.add)

                # ---------- cross-core exchange of (emax, gid) pairs ----------
                if exchange == "ar":
                    # masked AllReduce(add): core r contributes its pair only
                    # in cols (2r, 2r+1); the sum assembles the full 8-core
                    # table on every core, no transposes.
                    cc16 = sb1.tile([B, 2 * NC_N], F32, tag="cc16")
                    nc.vector.tensor_tensor(
                        out=cc16[:].rearrange("p (r t) -> p r t", r=NC_N),
                        in0=pay2[:].rearrange("p (o t) -> p o t", o=1)
                            .to_broadcast([B, NC_N, 2]),
                        in1=rmask16[:].rearrange("p (r t) -> p r t", r=NC_N),
                        op=OP.mult)
                    cc_in = dr.tile([B, 2 * NC_N], F32, tag="ccin")
                    cc_out = dr.tile([B, 2 * NC_N], F32, tag="ccout")
                    nc.sync.dma_start(cc_in[:], cc16[:])
                    if no_cc:
                        # perf probe only (wrong results): skip the collective
                        nc.sync.dma_start(cc_out[:], cc_in[:])
                    else:
                        nc.gpsimd.collective_compute(
                            "AllReduce", OP.add,
                            replica_groups=[list(range(NC_N))],
                            ins=[cc_in[:].opt()], outs=[cc_out[:].opt()])
                    ag16 = sb1.tile([B, 2 * NC_N], F32, tag="agsb")
                    nc.sync.dma_start(ag16[:], cc_out[:])
                    vag = ag16[:].rearrange("p (r t) -> p r t", r=NC_N)
                else:
                    # v1-style AllGather of a [1,128] row (pair-transposed)
                    payT_ps = tps.tile([128, 256], F32, tag="tp")
                    nc.tensor.transpose(payT_ps[0:1, 0:64], pay2[:, 0:1],
                                        ident[:])
                    nc.tensor.transpose(payT_ps[0:1, 64:128], pay2[:, 1:2],
                                        ident[:])
                    pay_row = sb1.tile([1, 128], F32, tag="payrow")
                    nc.vector.tensor_copy(pay_row[:], payT_ps[0:1, 0:128])
                    cc_in = dr.tile([1, 128], F32, tag="ccin")
                    cc_out = dr.tile([NC_N, 128], F32, tag="ccout")
                    # bounce DMAs MUST stay on the gpsimd queue: same-queue
                    # ordering with collective_compute is what serializes
                    # write->collective->read on hw.
                    nc.gpsimd.dma_start(cc_in[:], pay_row[:])
                    if no_cc:
                        for rr in range(NC_N):
                            nc.gpsimd.dma_start(cc_out[rr:rr + 1, :], cc_in[:])
                    else:
                        nc.gpsimd.collective_compute(
                            "AllGather", OP.bypass,
                            replica_groups=[list(range(NC_N))],
                            ins=[cc_in[:].opt()], outs=[cc_out[:].opt()])
                    ag_sb = sb1.tile([NC_N, 128], F32, tag="agsb")
                    nc.gpsimd.dma_start(ag_sb[:], cc_out[:])
                    agT_ps = tps.tile([128, 256], F32, tag="tp")
                    nc.tensor.transpose(agT_ps[0:B, 0:8], ag_sb[:, 0:64],
                                        ident[0:8, 0:8])
                    nc.tensor.transpose(agT_ps[0:B, 8:16], ag_sb[:, 64:128],
                                        ident[0:8, 0:8])
                    agT = sb1.tile([B, 16], F32, tag="agTsb")
                    nc.vector.tensor_copy(agT[:], agT_ps[0:B, 0:16])
                    # view cols (r, j): maxes at [:, 0:8], ids at [:, 8:16]
                    vag = agT[:].rearrange("p (t r) -> p r t", t=2)

                # ---------- global argmax ----------
                gm = sb1.tile([B, 1], F32, tag="gm")
                nc.vector.tensor_reduce(out=gm[:], in_=vag[:, :, 0:1].squeeze(),
                                        axis=AX.X, op=OP.max)
                mask = sb1.tile([B, 8], F32, tag="mask")
                nc.vector.tensor_scalar(out=mask[:], in0=vag[:, :, 0:1].squeeze(),
                                        scalar1=gm[:, 0:1], scalar2=None,
                                        op0=OP.is_lt)    # 1.0 where NOT max
                sel = sb1.tile([B, 8], F32, tag="sel")
                # sel = gid + (not-max)*BIG  -> min over ranks = winning gid
                nc.vector.tensor_scalar(out=sel[:], in0=mask[:],
                                        scalar1=2.0e9, scalar2=None,
                                        op0=OP.mult)
                nc.vector.tensor_tensor(out=sel[:], in0=sel[:],
                                        in1=vag[:, :, 1:2].squeeze(), op=OP.add)
                widf = sb1.tile([B, 1], F32, tag="widf")
                nc.vector.tensor_reduce(out=widf[:], in_=sel[:], axis=AX.X,
                                        op=OP.min)
                ids_i32 = sb1.tile([B, 1], I32, tag="ids")
                nc.vector.tensor_copy(ids_i32[:], widf[:])

                # ---------- feedback gather + xT ----------
                if not use_mtab:
                    x_sb = sb.tile([B, E], F32, tag="x")
                    nc.gpsimd.indirect_dma_start(
                        out=x_sb[:], out_offset=None, in_=d_emb,
                        in_offset=bass.IndirectOffsetOnAxis(ap=ids_i32[:, 0:1],
                                                            axis=0))
                    xT = sb.tile([128, KC * 64], F32, tag="xT")
                    transpose_to(xT, x_sb[:])

    nc.compile()
    return nc


_BUILT = {}


def _get_nc():
    key = (T, LOGITS_F32R, USE_MTAB)
    if key not in _BUILT:
        _BUILT[key] = build(T, LOGITS_F32R, USE_MTAB)
    return _BUILT[key]


def make_in_maps(z, emb, W_proj, b_proj, W_ih, b_ih, W_hh, b_hh, W_fc, b_fc):
    z = np.asarray(z, np.float32)
    emb = np.ascontiguousarray(np.asarray(emb, np.float32))
    W_proj = np.asarray(W_proj, np.float32)
    W_ih = np.asarray(W_ih, np.float32)
    W_hh = np.asarray(W_hh, np.float32)
    W_fc = np.asarray(W_fc, np.float32)
    b_proj = np.asarray(b_proj, np.float32)
    b_ih = np.asarray(b_ih, np.float32)
    b_hh = np.asarray(b_hh, np.float32)
    b_fc = np.asarray(b_fc, np.float32)

    wihT = np.ascontiguousarray(W_ih.T)            # [512, 1536]
    whhT = np.ascontiguousarray(W_hh.T)
    wprojT = np.ascontiguousarray(W_proj.T)        # [128, 512]
    zT = np.ascontiguousarray(z.T)                 # [128, 64]
    bias_gi = b_ih.copy()
    bias_gi[0:1024] += b_hh[0:1024]
    bias_gi = bias_gi[None, :]                     # [1, 1536]
    bias_hn = b_hh[None, 1024:1536]
    bias_proj = b_proj[None, :]

    common = dict(wihT=wihT, whhT=whhT, wprojT=wprojT, zT=zT,
                  bias_gi=bias_gi, bias_hn=bias_hn, bias_proj=bias_proj,
                  salt=np.zeros((1, max(1, int(SALT))), np.float32))
    if USE_MTAB:
        mtab = (emb @ W_ih.T + bias_gi).astype(np.float32)
        common["mtab"] = np.ascontiguousarray(mtab)
    else:
        common["emb"] = emb

    in_maps = []
    for c in range(NC_N):
        wfc_sh = W_fc[c * VS:(c + 1) * VS, :]          # [4000, 512]
        wfcT = np.zeros((H, VSP), np.float32)
        wfcT[:, 0:VS] = wfc_sh.T
        bias_fc = np.full((1, VSP), NEG, np.float32)
        bias_fc[0, 0:VS] = b_fc[c * VS:(c + 1) * VS]
        rank_col = np.full((B, 1), float(c * VS), np.float32)
        rank_mask16 = np.zeros((B, 2 * NC_N), np.float32)
        rank_mask16[:, 2 * c:2 * c + 2] = 1.0
        m = dict(common)
        m.update(wfcT=np.ascontiguousarray(wfcT), bias_fc=bias_fc,
                 rank_col=rank_col, rank_mask16=rank_mask16)
        if LOGITS_F32R:
            m["wb"] = np.ascontiguousarray(
                np.concatenate([wfc_sh, b_fc[c * VS:(c + 1) * VS, None]],
                               axis=1))
        in_maps.append(m)
    return in_maps


def kernel(z, emb, W_proj, b_proj, W_ih, b_ih, W_hh, b_hh, W_fc, b_fc,
           context_length):
    assert int(context_length) == T
    nc = _get_nc()
    in_maps = make_in_maps(z, emb, W_proj, b_proj, W_ih, b_ih, W_hh, b_hh,
                           W_fc, b_fc)
    res = bass_utils.run_bass_kernel_spmd(nc, in_maps,
                                          core_ids=list(range(NC_N)))
    shards = [res.results[c]["out"].reshape(B, T, VS) for c in range(NC_N)]
    return np.concatenate(shards, axis=2)

